# revision 46
# baseline (speedup 1.0000x reference)
"""BiTreeLSTM (ChildSum bottom-up + Chain top-down) over a complete binary tree,
depth 14 (16383 nodes), on 8 Trainium2 NeuronCores.

Sharding: per-level contiguous node sharding. Core k owns, for every level
l >= 3, the k-th contiguous 1/8 slice of that level's nodes. Children of core
k's nodes at level l are exactly core k's nodes at level l+1, so both
recursions are communication-free: each core returns its level-3 (h, c) plus
its phase-B running max, and the host finishes the 7-node tree top in fp32
(no on-device collective at all).

Compute layout: everything transposed ([feature, node]). Phase-A deep levels
and the phase-B i/f gate x-projections ride naive-fp8 DoubleRow GEMMs (2x PE
rate); phase-B o/u gates stay fp16 (the B max is sensitive to their noise).
The x-projection GEMM and the recurrence GEMM accumulate into the same PSUM
tile, so the big [N, 4*mem] intermediates are never materialized.
"""

import numpy as np

import concourse.bass as bass
import concourse.mybir as mybir
import concourse.tile as tile
from concourse import bacc
from concourse.bass_utils import run_bass_kernel_spmd

AFT = mybir.ActivationFunctionType
ALU = mybir.AluOpType
DR = mybir.MatmulPerfMode.DoubleRow
H = mybir.dt.float16
F32 = mybir.dt.float32
F8 = mybir.dt.float8e4

DEPTH = 14
IN = 1024
MEM = 512
NCORES = 8
KCX = IN // 128   # 8 contraction chunks for x projections
KCH = MEM // 128  # 4 contraction chunks for h projections
WS = 64.0         # host-side scale on fp8 weights (subnormal avoidance)
IWS = float(1.0 / WS)

# per-core column layout: cols 0..6 = global nodes 0..6 (replicated);
# then levels 3..13 contiguously (core-local slices)
L_OFF = {}
_off = 7
for _l in range(3, DEPTH):
    L_OFF[_l] = _off
    _off += 2 ** (_l - 3)
NCOLS = _off          # 2054
SH_COLS = L_OFF[11]   # 262: top7 + levels 3..10

# packed x layout: per-partition contiguous segments so every x DMA is
# 128 fat descriptors. fp16 pack: shallow + B-deep halves; fp8 pack: the
# deep 256-col chunks (consumed as naive-DoubleRow moving pairs by BOTH
# the phase-A deep cells and the phase-B deep i/f gates).
XO_SH = 0
_o = KCX * SH_COLS
XO_B = {}
for _lv, _c, _W in ((11, 0, 256), (12, 0, 512), (13, 0, 512), (13, 512, 512)):
    for _h in (0, 1):
        XO_B[(_lv, _c, _h)] = _o
        _o += (KCX // 2) * _W
XPACK_LEN = _o
XO_A = {}
_o = 0
for _lv, _c in ((13, 0), (13, 256), (13, 512), (13, 768), (12, 0),
                (12, 256), (11, 0)):
    XO_A[(_lv, _c)] = _o
    _o += KCX * 256
XPACK8_LEN = _o

# phase-A gate-chunk order j: i(0..3) o(4..7) u(8..11) f(12..15);
# wxa (ifoux) block layout is i,f,o,u -> block index for each j:
WXA_BLK = [0, 1, 2, 3, 8, 9, 10, 11, 12, 13, 14, 15, 4, 5, 6, 7]
# phase-B j order i(0..3) o(4..7) f(8..11) u(12..15).
# wxb8 packs i,f,o blocks (fp8): j -> block; wxbou packs u blocks (fp16 x64).
BXB8 = {j: p for p, j in enumerate([0, 1, 2, 3, 8, 9, 10, 11, 4, 5, 6, 7])}
BXB16 = {j: p for p, j in enumerate([12, 13, 14, 15])}

_PROG = None


def _bcast2(ap):
    """View [P, ..., N] as [P, ..., N, 2] with step 0 (each element twice)."""
    return bass.AP(tensor=ap.tensor, offset=ap.offset, ap=ap.ap + [[0, 2]])


def _dup_mid(ap):
    """View [P, ..., X] as [P, ..., 2, X] with step 0 on the new dim."""
    return bass.AP(tensor=ap.tensor, offset=ap.offset,
                   ap=ap.ap[:-1] + [[0, 2]] + [ap.ap[-1]])


def _pairs(ap):
    """(even, odd) views of the last dim interpreted as [..., t, 2]."""
    nd = len(ap.shape)
    letters = [chr(ord("a") + i) for i in range(nd - 1)]
    spec = " ".join(letters) + " (t two) -> " + " ".join(letters) + " t two"
    v = ap.rearrange(spec, two=2)
    idx = (slice(None),) * nd
    return v[idx + (0,)], v[idx + (1,)]


def build():
    """Build + compile the SPMD Bass program. Returns the Bacc object."""
    nc = bacc.Bacc("TRN2", target_bir_lowering=False, debug=False,
                   num_devices=NCORES)

    xpack = nc.dram_tensor("xpack", [128, XPACK_LEN], H,
                           kind="ExternalInput")
    xpack8 = nc.dram_tensor("xpack8", [128, XPACK8_LEN], F8,
                            kind="ExternalInput")
    xsh8h = nc.dram_tensor("xsh8h", [128, KCX * SH_COLS], F8,
                           kind="ExternalInput")
    xsh8l = nc.dram_tensor("xsh8l", [128, KCX * SH_COLS], F8,
                           kind="ExternalInput")
    # weights are host-packed partition-major ([128, kc*cols]) so every
    # load is a fat contiguous DMA -- strided-view loads cost 2-4us of
    # trigger time each on the issuing engine queue
    wxa8i = nc.dram_tensor("wxa8i", [128, KCX * MEM], F8,
                           kind="ExternalInput")
    wxa8ou = nc.dram_tensor("wxa8ou", [128, KCX * 2 * MEM], F8,
                            kind="ExternalInput")
    wxa8f = nc.dram_tensor("wxa8f", [128, KCX * MEM], F8,
                           kind="ExternalInput")
    wxa8l = nc.dram_tensor("wxa8l", [128, KCX * 4 * MEM], F8,
                           kind="ExternalInput")
    wha8 = nc.dram_tensor("wha8", [128, KCH * 3 * MEM], F8,
                          kind="ExternalInput")
    wfh8 = nc.dram_tensor("wfh8", [128, KCH * MEM], F8,
                          kind="ExternalInput")
    wha = nc.dram_tensor("wha", [128, KCH * 3 * MEM], H,
                         kind="ExternalInput")
    wfh = nc.dram_tensor("wfh", [128, KCH * MEM], H, kind="ExternalInput")
    wxb8 = nc.dram_tensor("wxb8", [128, KCX * 3 * MEM], F8,
                          kind="ExternalInput")
    wxbou = nc.dram_tensor("wxbou", [128, KCX * MEM], H,
                           kind="ExternalInput")
    whb8 = nc.dram_tensor("whb8", [128, KCH * 4 * MEM], F8,
                          kind="ExternalInput")
    ba = nc.dram_tensor("ba", [128, 16], F32, kind="ExternalInput")
    bb = nc.dram_tensor("bb", [128, 16], F32, kind="ExternalInput")
    bb64 = nc.dram_tensor("bb64", [128, 16], F32, kind="ExternalInput")
    sel3 = nc.dram_tensor("sel3", [128, 4], F32, kind="ExternalInput")
    ident = nc.dram_tensor("ident", [128, 128], F32, kind="ExternalInput")
    # out: [h3 | c3 | cmax] per core, host finishes the tree top
    out = nc.dram_tensor("out", [1, 3 * MEM], F32, kind="ExternalOutput")

    def xp_view(off, ln, kc):
        return xpack.ap()[:, off:off + ln].rearrange("p (kc n) -> p kc n",
                                                     kc=kc)

    pool_stack = []

    with tile.TileContext(nc) as tc:

        def open_pool(name, bufs=1, space="SBUF"):
            cm = tc.tile_pool(name=name, bufs=bufs, space=space)
            p = cm.__enter__()
            pool_stack.append((name, cm))
            return p

        def close_pool(name):
            n, cm = pool_stack.pop()
            assert n == name, f"pool close order: expected {n}, got {name}"
            cm.__exit__(None, None, None)

        persist = open_pool("persist")
        pp4 = open_pool("pp4", bufs=6, space="PSUM")
        pp2 = open_pool("pp2", bufs=2, space="PSUM")

        ba_sb = persist.tile([128, 16], F32, tag="ba")
        bb_sb = persist.tile([128, 16], F32, tag="bb")
        bb64_sb = persist.tile([128, 16], F32, tag="bb64")
        sel3_sb = persist.tile([128, 4], F32, tag="sel3")
        ident_sb = persist.tile([128, 128], F32, tag="ident")
        cmax = persist.tile([128, 4], F32, tag="cmax")
        stage = persist.tile([128, 12], F32, tag="stage")
        wxa8_t = persist.tile([128, KCX, 2, 4 * MEM], F8, tag="wxa8",
                              name="wxa8")
        wxb8_t = persist.tile([128, KCX, 3 * MEM], F8, tag="wxb8",
                              name="wxb8")
        wxbou_t = [persist.tile([128, MEM], H, tag=f"wxbou{kc}",
                                name=f"wxbou{kc}") for kc in range(KCX)]
        wha_sb = persist.tile([128, KCH, 3 * MEM], H, tag="wha")
        wfh_sb = persist.tile([128, KCH, MEM], H, tag="wfh")
        whb_sb = persist.tile([128, KCH, 4 * MEM], F8, tag="whb")
        xTsh = persist.tile([128, KCX, SH_COLS], H, tag="xTsh")
        xTsh8 = persist.tile([128, KCX, 2, SH_COLS], F8, tag="xTsh8",
                             name="xTsh8")
        xgshB = persist.tile([128, 16, SH_COLS], H, tag="xgshB")

        nc.gpsimd.dma_start(out=ba_sb[:], in_=ba.ap())
        nc.gpsimd.dma_start(out=bb64_sb[:], in_=bb64.ap())
        nc.gpsimd.dma_start(out=bb_sb[:], in_=bb.ap())
        nc.vector.memset(cmax[:], -3.0e38)

        # ============ helpers ============

        def mm_xa8(ps, xt8, blk, start=True, stop=True):
            """deep-A x-projection: naive fp8 DoubleRow over kc pairs."""
            c0, c1 = blk * 128, (blk + 1) * 128
            for q0 in range(0, KCX, 2):
                nc.tensor.matmul(ps, wxa8_t[:, q0:q0 + 2, 0, c0:c1],
                                 xt8[:, q0:q0 + 2, :],
                                 start=(start and q0 == 0),
                                 stop=(stop and q0 + 2 >= KCX), perf_mode=DR)

        def mm_xa15(ps, xt8, blk, start=True, stop=True):
            """shallow-A x-projection: 1.5-term compensated fp8."""
            c0, c1 = blk * 128, (blk + 1) * 128
            for q in range(KCX):
                nc.tensor.matmul(ps, wxa8_t[:, q, :, c0:c1],
                                 _dup_mid(xt8[:, q, 0, :]),
                                 start=(start and q == 0), stop=False,
                                 perf_mode=DR)
            for q0 in range(0, KCX, 2):
                nc.tensor.matmul(ps, wxa8_t[:, q0:q0 + 2, 0, c0:c1],
                                 xt8[:, q0:q0 + 2, 1, :], start=False,
                                 stop=(stop and q0 + 2 >= KCX), perf_mode=DR)

        def mm_ha8(ps, w8, h8, blk, sl, start=True, stop=True):
            """deep-A recurrence: naive fp8 DoubleRow over kc pairs."""
            c0, c1 = blk * 128, (blk + 1) * 128
            for q0 in range(0, KCH, 2):
                nc.tensor.matmul(ps, w8[:, q0:q0 + 2, c0:c1],
                                 h8[:, q0:q0 + 2, sl],
                                 start=(start and q0 == 0),
                                 stop=(stop and q0 + 2 >= KCH), perf_mode=DR)

        def mm_h8(ps, h8, blk, sl, start=True, stop=True):
            """chain recurrence on whb8. DoubleRow for wide moving dims;
            plain per-chunk matmuls below FD=64 (DR's LDWEIGHTS overhead
            loses there)."""
            c0, c1 = blk * 128, (blk + 1) * 128
            fd = sl.stop - sl.start
            if fd >= 64:
                for q0 in range(0, KCH, 2):
                    nc.tensor.matmul(ps, whb_sb[:, q0:q0 + 2, c0:c1],
                                     h8[:, q0:q0 + 2, sl],
                                     start=(start and q0 == 0),
                                     stop=(stop and q0 + 2 >= KCH),
                                     perf_mode=DR)
            else:
                for q in range(KCH):
                    nc.tensor.matmul(ps, whb_sb[:, q, c0:c1], h8[:, q, sl],
                                     start=(start and q == 0),
                                     stop=(stop and q == KCH - 1))

        def mm_bd8(ps, segs, j, c0, h8d):
            """phase-B deep i/f/o gate psum: per 256-col half, the naive-fp8
            x-projection opens the accumulation and that half's recurrence
            GEMM (parent-duplicated fp8 h) closes it."""
            blk = BXB8[j]
            x0, x1 = blk * 128, (blk + 1) * 128
            r0, r1 = j * 128, (j + 1) * 128
            for s, seg in enumerate(segs):
                o = s * 256
                for q0 in range(0, KCX, 2):
                    nc.tensor.matmul(ps[:, o:o + 256],
                                     wxb8_t[:, q0:q0 + 2, x0:x1],
                                     seg[:, q0:q0 + 2, :],
                                     start=q0 == 0, stop=False,
                                     perf_mode=DR)
                for q0 in range(0, KCH, 2):
                    nc.tensor.matmul(ps[:, o:o + 256],
                                     whb_sb[:, q0:q0 + 2, r0:r1],
                                     h8d[:, q0:q0 + 2, c0 + o:c0 + o + 256],
                                     start=False, stop=q0 + 2 >= KCH,
                                     perf_mode=DR)

        def mm_bd16(ps, xt0, xt1, j, c0, W, h8d):
            """phase-B deep u gate psum: fp16 x-projection + fp8 recurrence,
            per 256-col half as in mm_bd8."""
            blk = BXB16[j]
            x0, x1 = blk * 128, (blk + 1) * 128
            r0, r1 = j * 128, (j + 1) * 128
            for o in range(0, W, 256):
                for kc in range(KCX):
                    xt_sl = (xt0[:, kc, o:o + 256] if kc < KCX // 2
                             else xt1[:, kc - KCX // 2, o:o + 256])
                    nc.tensor.matmul(ps[:, o:o + 256],
                                     wxbou_t[kc][:, x0:x1], xt_sl,
                                     start=kc == 0, stop=False)
                for q0 in range(0, KCH, 2):
                    nc.tensor.matmul(ps[:, o:o + 256],
                                     whb_sb[:, q0:q0 + 2, r0:r1],
                                     h8d[:, q0:q0 + 2, c0 + o:c0 + o + 256],
                                     start=False, stop=q0 + 2 >= KCH,
                                     perf_mode=DR)

        def alloc_hc(pool, M, with_c=True):
            Mp = max(M, 2)  # matmul moving dim must be >= 2; pad tiny levels
            h = pool.tile([128, KCH, Mp], H, tag=f"h{M}", bufs=1,
                          name=f"h{M}")
            c = (pool.tile([128, KCH, Mp], F32, tag=f"c{M}", bufs=1,
                           name=f"c{M}") if with_c else None)
            # pad columns are never read (their matmul psum outputs are
            # never consumed), so they stay uninitialized
            return h, c

        def csum_cell(pool, W, xt8, hs8, chh8, c_pv, cb, h_dst,
                      c_dst, d0, wha8_t, wfh8_t, leaf=False):
            """Deep child-sum cell, all-fp8 naive DoubleRow GEMMs (64x psums).

            xt8: [128, KCX, W] fp8 x chunk; hs8: [128, KCH, W] fp8 child
            h-sum; chh8: [128, KCH, 2W] fp8 interleaved children h."""
            for mc in range(4):
                gates = {}
                for gi, j in (("i", mc), ("o", 4 + mc), ("u", 8 + mc)):
                    ps = pp4.tile([128, W], F32, tag="ps_g", name="psg")
                    mm_xa8(ps[:], xt8, WXA_BLK[j], start=True, stop=leaf)
                    if not leaf:
                        mm_ha8(ps[:], wha8_t, hs8, j, slice(0, W),
                               start=False, stop=True)
                    g = pool.tile([128, W], H, tag=f"g_{gi}", name="g")
                    nc.scalar.activation(
                        out=g[:], in_=ps[:],
                        func=AFT.Tanh if gi == "u" else AFT.Sigmoid,
                        bias=ba_sb[:, j:j + 1], scale=IWS)
                    gates[gi] = g
                c_sl = c_dst[:, mc, d0:d0 + W]
                h_sl = h_dst[:, mc, d0:d0 + W]
                if leaf:
                    nc.vector.tensor_mul(c_sl, gates["i"][:], gates["u"][:])
                else:
                    psx = pp4.tile([128, W], F32, tag="ps_g", name="psx")
                    mm_xa8(psx[:], xt8, WXA_BLK[12 + mc])
                    fx = pool.tile([128, W], F32, tag="fx", bufs=1, name="fx")
                    nc.scalar.activation(out=fx[:], in_=psx[:],
                                         func=AFT.Identity,
                                         bias=ba_sb[:, 12 + mc:13 + mc],
                                         scale=IWS)
                    psF = pp2.tile([128, 2 * W], F32, tag="ps_F", name="psF")
                    mm_ha8(psF[:], wfh8_t, chh8, mc, slice(0, 2 * W))
                    f_sb = pool.tile([128, 2 * W], H, tag="f", bufs=1,
                                     name="f")
                    nc.vector.scalar_tensor_tensor(
                        f_sb[:].rearrange("p (t two) -> p t two", two=2),
                        psF[:].rearrange("p (t two) -> p t two", two=2),
                        IWS, _bcast2(fx[:]), op0=ALU.mult, op1=ALU.add)
                    nc.scalar.activation(out=f_sb[:], in_=f_sb[:],
                                         func=AFT.Sigmoid)
                    nc.vector.tensor_mul(c_sl, gates["i"][:], gates["u"][:])
                    f_e, f_o = _pairs(f_sb[:])
                    c_e, c_o = _pairs(c_pv[:, mc, cb:cb + 2 * W])
                    t1 = pool.tile([128, W], H, tag="t1", bufs=1, name="t1")
                    nc.vector.tensor_mul(t1[:], f_e, c_e)
                    nc.vector.tensor_add(c_sl, c_sl, t1[:])
                    t2 = pool.tile([128, W], H, tag="t2", bufs=1, name="t2")
                    nc.vector.tensor_mul(t2[:], f_o, c_o)
                    nc.vector.tensor_add(c_sl, c_sl, t2[:])
                th = pool.tile([128, W], H, tag="th", bufs=1, name="th")
                nc.scalar.activation(out=th[:], in_=c_sl, func=AFT.Tanh)
                nc.vector.tensor_mul(h_sl, gates["o"][:], th[:])

        def phaseA_level(pool, hc_pool, M, xg_off, xg_t, h_pv, c_pv,
                         wha_t, wfh_t, ppg=None, ppF=None,
                         gtag="ps_g"):
            """Shallow child-sum level from precomputed x pre-activations."""
            ppg = ppg or pp4
            ppF = ppF or pp2
            h_cur, c_cur = alloc_hc(hc_pool, M)
            Mp = max(M, 2)
            hs = pool.tile([128, KCH, Mp], H, tag="hsumS", bufs=1, name="hs")
            he, ho = _pairs(h_pv[:, :, 0:2 * M])
            nc.vector.tensor_add(hs[:, :, 0:M], he, ho)
            gate = pool.tile([128, 12, M], H, tag="gateA", bufs=1,
                             name="gate")
            gsz = max(1, 512 // Mp)
            for g0 in range(0, 12, gsz):
                g1 = min(12, g0 + gsz)
                ps = ppg.tile([128, g1 - g0, Mp], F32, tag=gtag, name="psg")
                for j in range(g0, g1):
                    for kc in range(KCH):
                        nc.tensor.matmul(
                            ps[:, j - g0, :],
                            wha_t[:, kc, j * 128:(j + 1) * 128],
                            hs[:, kc, :], start=kc == 0, stop=kc == KCH - 1)
                pre = pool.tile([128, g1 - g0, M], F32, tag="preA", name="pre")
                nc.vector.tensor_add(pre[:], xg_t[:, g0:g1, xg_off:xg_off + M],
                                     ps[:, :, 0:M])
                if g0 < 8:
                    s1 = min(g1, 8)
                    nc.scalar.activation(out=gate[:, g0:s1, :],
                                         in_=pre[:, 0:s1 - g0, :],
                                         func=AFT.Sigmoid)
                if g1 > 8:
                    s0 = max(g0, 8)
                    nc.scalar.activation(out=gate[:, s0:g1, :],
                                         in_=pre[:, s0 - g0:g1 - g0, :],
                                         func=AFT.Tanh)
            fga = pool.tile([128, 4, 2 * M], H, tag="fgA", bufs=1,
                            name="fga")
            gF = max(1, 512 // (2 * M))
            for m0 in range(0, 4, gF):
                m1 = min(4, m0 + gF)
                psF = ppF.tile([128, m1 - m0, 2 * M], F32, tag="ps_F",
                               name="psF")
                for mc in range(m0, m1):
                    for kc in range(KCH):
                        nc.tensor.matmul(
                            psF[:, mc - m0, :],
                            wfh_t[:, kc, mc * 128:(mc + 1) * 128],
                            h_pv[:, kc, 0:2 * M], start=kc == 0,
                            stop=kc == KCH - 1)
                fxv = xg_t[:, 12 + m0:12 + m1, xg_off:xg_off + M]
                nc.vector.tensor_add(
                    fga[:, m0:m1, :].rearrange("p m (t two) -> p m t two",
                                               two=2),
                    psF[:].rearrange("p m (t two) -> p m t two", two=2),
                    _bcast2(fxv))
            nc.scalar.activation(out=fga[:], in_=fga[:], func=AFT.Sigmoid)
            fe, fo = _pairs(fga[:])
            ce, co = _pairs(c_pv[:, :, 0:2 * M])
            c_sl = c_cur[:, :, 0:M]
            h_sl = h_cur[:, :, 0:M]
            nc.vector.tensor_mul(c_sl, gate[:, 0:4, :], gate[:, 8:12, :])
            t1 = pool.tile([128, 4, M], H, tag="t1A", bufs=1, name="t1")
            nc.vector.tensor_mul(t1[:], fe, ce)
            nc.vector.tensor_add(c_sl, c_sl, t1[:])
            t2 = pool.tile([128, 4, M], H, tag="t2A", bufs=1, name="t2")
            nc.vector.tensor_mul(t2[:], fo, co)
            nc.vector.tensor_add(c_sl, c_sl, t2[:])
            th = pool.tile([128, 4, M], H, tag="thA", bufs=1, name="th")
            nc.scalar.activation(out=th[:], in_=c_sl, func=AFT.Tanh)
            nc.vector.tensor_mul(h_sl, gate[:, 4:8, :], th[:])
            return h_cur, c_cur

        # static pool for the final-output transpose
        pT = open_pool("pTop", bufs=1)

        # ============ phase A ============

        def chain_update(pool, h_ap, M, mc=None, eng=None):
            eng = eng or nc.vector
            if mc is not None:  # single mem-chunk slab [128, M]
                rm = pool.tile([128, 1], F32, tag="rmax", bufs=1, name="rm")
                eng.tensor_reduce(out=rm[:], in_=h_ap,
                                  axis=mybir.AxisListType.X, op=ALU.max)
                eng.tensor_max(cmax[:, mc:mc + 1], cmax[:, mc:mc + 1],
                               rm[:])
            elif M == 1:
                rm1 = pool.tile([128, KCH], F32, tag="rmaxq", bufs=1,
                                name="rm1")
                eng.tensor_copy(out=rm1[:], in_=h_ap[:, :, 0])
                eng.tensor_max(cmax[:], cmax[:], rm1[:])
            else:
                rm = pool.tile([128, KCH], F32, tag="rmaxq", bufs=1, name="rm")
                eng.tensor_reduce(out=rm[:], in_=h_ap,
                                  axis=mybir.AxisListType.X, op=ALU.max)
                eng.tensor_max(cmax[:], cmax[:], rm[:])

        def phaseB_level(pool, M, xg_off, h8_pv, c_pv, direct=False,
                         root=False, dup=False, need_h8=True):
            """Chain level. h8_pv: fp8 parent h [128, KCH, Pp].
            Returns (h_cur, c_cur, h8_cur); h8_cur is child-duplicated
            [128, KCH, 2M] when dup=True (feeds the B-deep recurrence)."""
            h_cur, c_cur = alloc_hc(hcB, M)
            Mp = max(M, 2)
            h8_cur = None
            if need_h8:
                h8_cur = hcB.tile([128, KCH, (2 if dup else 1) * Mp], F8,
                                  tag=f"h8b{M}{'d' if dup else ''}", bufs=1,
                                  name=f"h8b{M}")
            Pn = M if direct else M // 2
            Pp = max(Pn, 2)
            gate = pool.tile([128, 16, M], H, tag="gateB", bufs=1,
                             name="gate")
            if root:
                nc.scalar.activation(out=gate[:, 0:12, :],
                                     in_=xgshB[:, 0:12, xg_off:xg_off + M],
                                     func=AFT.Sigmoid, scale=IWS)
                nc.scalar.activation(out=gate[:, 12:16, :],
                                     in_=xgshB[:, 12:16, xg_off:xg_off + M],
                                     func=AFT.Tanh, scale=IWS)
            else:
                pre = pool.tile([128, 16, M], H, tag="preB", bufs=1,
                                name="pre")
                gsz = max(1, 512 // Pp)
                for g0 in range(0, 16, gsz):
                    g1 = min(16, g0 + gsz)
                    ps = pp4.tile([128, g1 - g0, Pp], F32, tag="ps_g",
                                  name="psg")
                    for j in range(g0, g1):
                        mm_h8(ps[:, j - g0, :], h8_pv, j, slice(0, Pp))
                    xgv = xgshB[:, g0:g1, xg_off:xg_off + M]
                    psv = ps[:, :, 0:Pn]
                    if direct:
                        nc.vector.tensor_add(pre[:, g0:g1, :], xgv, psv)
                    else:
                        nc.vector.tensor_add(
                            pre[:, g0:g1, :].rearrange(
                                "p q (t two) -> p q t two", two=2),
                            xgv.rearrange("p q (t two) -> p q t two", two=2),
                            _bcast2(psv))
                nc.scalar.activation(out=gate[:, 0:12, :], in_=pre[:, 0:12, :],
                                     func=AFT.Sigmoid, scale=IWS)
                nc.scalar.activation(out=gate[:, 12:16, :],
                                     in_=pre[:, 12:16, :], func=AFT.Tanh,
                                     scale=IWS)
            c_sl = c_cur[:, :, 0:M]
            h_sl = h_cur[:, :, 0:M]
            nc.vector.tensor_mul(c_sl, gate[:, 0:4, :], gate[:, 12:16, :])
            if not root:
                if direct:
                    t1 = pool.tile([128, 4, M], H, tag="t1B", bufs=1,
                                   name="t1")
                    nc.vector.tensor_mul(t1[:], gate[:, 8:12, :],
                                         c_pv[:, :, 0:Pn])
                    nc.vector.tensor_add(c_sl, c_sl, t1[:])
                else:
                    fe, fo = _pairs(gate[:, 8:12, :])
                    ce, co = _pairs(c_sl)
                    t1 = pool.tile([128, 4, Pn], H, tag="t1B", bufs=1,
                                   name="t1")
                    nc.vector.tensor_mul(t1[:], fe, c_pv[:, :, 0:Pn])
                    nc.vector.tensor_add(ce, ce, t1[:])
                    t2 = pool.tile([128, 4, Pn], H, tag="t2B", bufs=1,
                                   name="t2")
                    nc.vector.tensor_mul(t2[:], fo, c_pv[:, :, 0:Pn])
                    nc.vector.tensor_add(co, co, t2[:])
            th = pool.tile([128, 4, M], H, tag="thB", bufs=1, name="th")
            nc.scalar.activation(out=th[:], in_=c_sl, func=AFT.Tanh)
            nc.vector.tensor_mul(h_sl, gate[:, 4:8, :], th[:])
            if dup:
                nc.vector.tensor_copy(
                    out=h8_cur[:, :, 0:2 * M].rearrange(
                        "p q (t two) -> p q t two", two=2),
                    in_=_bcast2(h_sl))
            elif need_h8:
                nc.vector.tensor_copy(out=h8_cur[:, :, 0:M], in_=h_sl)
            chain_update(pool, h_cur[:, :, 0:M], M)
            return h_cur, c_cur, h8_cur

        def emit_b3_parent(h_pv, c_pv):
            hpar = pBs.tile([128, KCH, 2], H, tag="hpar", bufs=1)
            hpar8 = pBs.tile([128, KCH, 2], F8, tag="hpar8", bufs=1)
            cpar = pBs.tile([128, KCH, 2], F32, tag="cpar", bufs=1)
            for c in range(4):
                if c == 0:
                    nc.vector.tensor_scalar_mul(hpar[:, :, 0:1],
                                                h_pv[:, :, c:c + 1],
                                                sel3_sb[:, c:c + 1])
                    nc.vector.tensor_scalar_mul(cpar[:, :, 0:1],
                                                c_pv[:, :, c:c + 1],
                                                sel3_sb[:, c:c + 1])
                else:
                    nc.vector.scalar_tensor_tensor(
                        hpar[:, :, 0:1], h_pv[:, :, c:c + 1],
                        sel3_sb[:, c:c + 1], hpar[:, :, 0:1],
                        op0=ALU.mult, op1=ALU.add)
                    nc.vector.scalar_tensor_tensor(
                        cpar[:, :, 0:1], c_pv[:, :, c:c + 1],
                        sel3_sb[:, c:c + 1], cpar[:, :, 0:1],
                        op0=ALU.mult, op1=ALU.add)
            nc.vector.tensor_copy(out=hpar8[:, :, 0:1], in_=hpar[:, :, 0:1])
            return hpar8, cpar

        hcB = open_pool("hcB")
        pBs = open_pool("pBsh", bufs=2)
        hcA = open_pool("hcA")
        hcA12 = open_pool("hcA12")
        wha8_sb = hcA12.tile([128, KCH, 3 * MEM], F8, tag="wha8")
        wfh8_sb = hcA12.tile([128, KCH, MEM], F8, tag="wfh8")
        pAd = open_pool("pAdeep", bufs=2)

        # DMA order: the first leaf cell's operands (xt chunks + wxa8h
        # i/o/u blocks, ~2.3MB) go first on sync so the PE starts ASAP;
        # the B-shallow xg batch operands (xTsh/wxb8/xsh8-hi/wxbou) stream
        # on scalar meanwhile.
        def xp8_view(off, ln):
            return xpack8.ap()[:, off:off + ln].rearrange(
                "p (kc n) -> p kc n", kc=KCX)

        # NO DMA triggers ride the scalar queue: gate activations live
        # there and a ring-credit-throttled trigger backlog delays them.
        # sync = leaf-critical loads; gpsimd (otherwise idle) = the rest.
        xt_pre = []
        for g in range(2):
            xt = pAd.tile([128, KCX, 256], F8, tag="xt", name="xt",
                          bufs=4)
            nc.sync.dma_start(out=xt[:],
                              in_=xp8_view(XO_A[(13, g * 256)], KCX * 256))
            xt_pre.append(xt)
        nc.sync.dma_start(
            out=wxa8_t[:, :, 0, 0:MEM],
            in_=wxa8i.ap().rearrange("p (kc m) -> p kc m", kc=KCX))
        wxa8ou_r = wxa8ou.ap().rearrange("p (kc m) -> p kc m", kc=KCX)
        nc.sync.dma_start(out=wxa8_t[:, :, 0, 2 * MEM:3 * MEM],
                          in_=wxa8ou_r[:, :, 0:MEM])
        nc.sync.dma_start(out=wxa8_t[:, :, 0, 3 * MEM:4 * MEM],
                          in_=wxa8ou_r[:, :, MEM:2 * MEM])
        nc.gpsimd.dma_start(
            out=xTsh8[:, :, 0, :],
            in_=xsh8h.ap().rearrange("p (kc n) -> p kc n", kc=KCX))
        nc.gpsimd.dma_start(
            out=wxb8_t[:],
            in_=wxb8.ap().rearrange("p (kc m) -> p kc m", kc=KCX))
        nc.gpsimd.dma_start(out=xTsh[:],
                            in_=xp_view(XO_SH, KCX * SH_COLS, KCX))
        for kc in range(KCX):
            [nc.gpsimd, nc.sync][kc % 2].dma_start(
                out=wxbou_t[kc][:],
                in_=wxbou.ap()[:, kc * MEM:(kc + 1) * MEM])
        nc.sync.dma_start(
            out=wxa8_t[:, :, 0, MEM:2 * MEM],
            in_=wxa8f.ap().rearrange("p (kc m) -> p kc m", kc=KCX))
        nc.sync.dma_start(
            out=wha8_sb[:],
            in_=wha8.ap().rearrange("p (kc m) -> p kc m", kc=KCH))
        nc.sync.dma_start(
            out=wfh8_sb[:],
            in_=wfh8.ap().rearrange("p (kc m) -> p kc m", kc=KCH))
        nc.gpsimd.dma_start(
            out=whb_sb[:],
            in_=whb8.ap().rearrange("p (kc m) -> p kc m", kc=KCH))
        nc.gpsimd.dma_start(
            out=wha_sb[:],
            in_=wha.ap().rearrange("p (kc m) -> p kc m", kc=KCH))
        nc.gpsimd.dma_start(
            out=wfh_sb[:],
            in_=wfh.ap().rearrange("p (kc m) -> p kc m", kc=KCH))
        nc.gpsimd.dma_start(
            out=xTsh8[:, :, 1, :],
            in_=xsh8l.ap().rearrange("p (kc n) -> p kc n", kc=KCX))
        nc.gpsimd.dma_start(
            out=wxa8_t[:, :, 1, :],
            in_=wxa8l.ap().rearrange("p (kc m) -> p kc m", kc=KCX))
        nc.gpsimd.dma_start(out=sel3_sb[:], in_=sel3.ap())
        nc.gpsimd.dma_start(out=ident_sb[:], in_=ident.ap())

        # phase-B xg batch: i/f/o blocks via naive fp8 DR; u via fp16 (x64
        # weights) -- every psum lands directly in xgshB's 64x convention.
        def emit_bxg_fp8():
            for j in (0, 1, 2, 3, 8, 9, 10, 11, 4, 5, 6, 7):
                ps = pp2.tile([128, SH_COLS], F32, tag="ps_F", name="psb")
                blk = BXB8[j]
                for q0 in range(0, KCX, 2):
                    nc.tensor.matmul(ps[:],
                                     wxb8_t[:, q0:q0 + 2,
                                            blk * 128:(blk + 1) * 128],
                                     xTsh8[:, q0:q0 + 2, 0, :],
                                     start=q0 == 0, stop=q0 + 2 >= KCX,
                                     perf_mode=DR)
                nc.scalar.activation(out=xgshB[:, j, :], in_=ps[:],
                                     func=AFT.Identity,
                                     bias=bb64_sb[:, j:j + 1])

        def emit_bxg_u():
            for j in (12, 13, 14, 15):
                ps = pp2.tile([128, SH_COLS], F32, tag="ps_F", name="psb")
                blk = BXB16[j]
                for kc in range(KCX):
                    nc.tensor.matmul(ps[:],
                                     wxbou_t[kc][:, blk * 128:(blk + 1) * 128],
                                     xTsh[:, kc, :], start=kc == 0,
                                     stop=kc == KCX - 1)
                nc.scalar.activation(out=xgshB[:, j, :], in_=ps[:],
                                     func=AFT.Identity,
                                     bias=bb64_sb[:, j:j + 1])

        # B-chain emitter: one item per call, sprinkled between A-deep units
        b_state = {"h": None, "c": None, "h8": None, "idx": 0}

        def emit_b_item():
            i = b_state["idx"]
            b_state["idx"] += 1
            if i > 10:
                return
            if i == 0:
                (b_state["h"], b_state["c"],
                 b_state["h8"]) = phaseB_level(pBs, 1, 0, None, None,
                                               root=True)
            elif i in (1, 2):
                m, xo = (2, 1) if i == 1 else (4, 3)
                (b_state["h"], b_state["c"],
                 b_state["h8"]) = phaseB_level(pBs, m, xo, b_state["h8"],
                                               b_state["c"],
                                               need_h8=(i == 1))
            elif i == 3:
                hpar8, cpar = emit_b3_parent(b_state["h"], b_state["c"])
                (b_state["h"], b_state["c"],
                 b_state["h8"]) = phaseB_level(pBs, 1, L_OFF[3], hpar8, cpar,
                                               direct=True)
            else:
                (b_state["h"], b_state["c"],
                 b_state["h8"]) = phaseB_level(pBs, 2 ** (i - 3), L_OFF[i],
                                               b_state["h8"], b_state["c"],
                                               dup=(i == 10))

        def make_hs8_chh8(h_pv, W2):
            """fp8 child-sum + interleaved children from fp16 h_pv."""
            hs8 = pAd.tile([128, KCH, W2 // 2], F8, tag="hs8d", bufs=1,
                           name="hs8")
            he, ho = _pairs(h_pv[:, :, 0:W2])
            nc.vector.tensor_add(hs8[:], he, ho)
            chh8 = pAd.tile([128, KCH, W2], F8, tag="chh8d", bufs=1,
                            name="chh8")
            nc.vector.tensor_copy(out=chh8[:], in_=h_pv[:, :, 0:W2])
            return hs8, chh8

        # levels 13+12 fused: leaf chunks are consumed immediately; the fp8
        # casts for the level-12 recurrence are emitted per leaf chunk so
        # they overlap the next chunk's compute. The B xg batch is emitted
        # after the first leaf cell (its 6MB of operands stream while the
        # PE chews the ~2MB leaf working set).
        def load_xt_a(lv, base):
            xt = pAd.tile([128, KCX, 256], F8, tag="xt", bufs=4)
            nc.sync.dma_start(out=xt[:],
                              in_=xp8_view(XO_A[(lv, base)], KCX * 256))
            return xt

        h12, c12 = alloc_hc(hcA12, 512)
        xt_q = {}
        for c0 in (0, 256):
            h13c = pAd.tile([128, KCH, 512], H, tag="h13c", bufs=1)
            c13c = pAd.tile([128, KCH, 512], F32, tag="c13c", bufs=1)
            hs8 = pAd.tile([128, KCH, 256], F8, tag="hs8d", bufs=1,
                           name="hs8")
            chh8 = pAd.tile([128, KCH, 512], F8, tag="chh8d", bufs=1,
                            name="chh8")
            # prefetch this slab's l12 chunk ahead of the leaf compute so
            # the gpsimd DMA queue never gates the level-12 cell
            xt_q[(12, c0)] = load_xt_a(12, c0)
            for sc in (0, 1):
                base = 2 * c0 + sc * 256
                if c0 == 0:
                    xt = xt_pre[sc]
                else:
                    xt = xt_q[(13, base)]
                csum_cell(pAd, 256, xt, None, None, None, 0,
                          h13c, c13c, sc * 256, wha8_sb, wfh8_sb, leaf=True)
                sl = slice(sc * 256, sc * 256 + 256)
                he, ho = _pairs(h13c[:, :, sl])
                nc.vector.tensor_add(hs8[:, :, sc * 128:sc * 128 + 128],
                                     he, ho)
                nc.vector.tensor_copy(out=chh8[:, :, sl],
                                      in_=h13c[:, :, sl])
                if c0 == 0 and sc == 0:
                    emit_bxg_fp8()
                    xt_q[(13, 512)] = load_xt_a(13, 512)
                    xt_q[(13, 768)] = load_xt_a(13, 768)
                elif c0 == 0 and sc == 1:
                    emit_bxg_u()
                else:
                    emit_b_item()
            csum_cell(pAd, 256, xt_q[(12, c0)], hs8, chh8, c13c, 0,
                      h12, c12, c0, wha8_sb, wfh8_sb)
            emit_b_item()
            emit_b_item()
            if c0 == 0:
                xt_q[(11, 0)] = load_xt_a(11, 0)
        # level 11
        h11, c11 = alloc_hc(hcA, 256)
        hs8, chh8 = make_hs8_chh8(h12, 512)
        csum_cell(pAd, 256, xt_q[(11, 0)], hs8, chh8, c12, 0,
                  h11, c11, 0, wha8_sb, wfh8_sb)
        while b_state["idx"] <= 10:
            emit_b_item()
        close_pool("pAdeep")
        close_pool("hcA12")

        # xg batch for shallow cols (phase A)
        pBd = open_pool("pBdeep", bufs=2)
        pBatchA = open_pool("pBatchA", bufs=2)
        xgshA = pBatchA.tile([128, 16, SH_COLS], H, tag="xgsh", bufs=1)
        for j in range(16):
            ps = pp2.tile([128, SH_COLS], F32, tag="ps_F", name="psb")
            if 8 <= j < 12:  # u gate needs the 1.5-term compensation
                mm_xa15(ps[:], xTsh8, WXA_BLK[j])
            else:  # i, o, f tolerate naive fp8 (sim: relA ~1e-2)
                mm_xa8(ps[:], xTsh8[:, :, 0, :], WXA_BLK[j])
            nc.scalar.activation(out=xgshA[:, j, :], in_=ps[:],
                                 func=AFT.Identity, bias=ba_sb[:, j:j + 1],
                                 scale=IWS)

        # ---- B-deep emitters (interleaved with A-shallow below) ----
        bd = {"h8": b_state["h8"], "c": b_state["c"], "xt": None}

        def bdeep_load_xt(l, c0, W):
            # all triggers ride the sync queue: a trigger stalled on a
            # buffer-free would block its queue, and gpsimd carries the
            # latency-critical h8 dup copies for the level chain
            xt0 = pBd.tile([128, KCX // 2, W], H, tag="xtB", bufs=4,
                           name="xt0")
            xt1 = pBd.tile([128, KCX // 2, W], H, tag="xtB", bufs=4,
                           name="xt1")
            nc.sync.dma_start(
                out=xt0[:], in_=xp_view(XO_B[(l, c0, 0)], KCX // 2 * W,
                                        KCX // 2))
            nc.sync.dma_start(
                out=xt1[:], in_=xp_view(XO_B[(l, c0, 1)], KCX // 2 * W,
                                        KCX // 2))
            segs = []
            for s in range(W // 256):
                t8 = pBd.tile([128, KCX, 256], F8, tag="xt8B", bufs=4,
                              name="xt8")
                nc.sync.dma_start(
                    out=t8[:],
                    in_=xp8_view(XO_A[(l, c0 + s * 256)], KCX * 256))
                segs.append(t8)
            return (xt0, xt1, segs)

        def bdeep_mc(l, c0, mc, W, hb, h8b, cb, h8_pv, c_pv):
            last = l == 13
            xt0, xt1, segs = bd["xt"]
            # i, f, o gates: naive fp8; u: fp16 (x64 weights). All psums are
            # 64x; the recurrence GEMM accumulates into the x-projection psum
            # (parent h stored child-duplicated) so each gate activation
            # reads ONE finished psum with bias + 1/64 scale in one shot.
            g3 = pBd.tile([128, 3, W], H, tag="g3D", bufs=2, name="g3")
            for gn, j in ((0, mc), (1, 8 + mc), (2, 4 + mc)):  # i, f, o
                psx = pp4.tile([128, W], F32, tag="ps_g", name="psx")
                mm_bd8(psx[:], segs, j, c0, h8_pv)
                nc.scalar.activation(out=g3[:, gn, :], in_=psx[:],
                                     func=AFT.Sigmoid,
                                     bias=bb_sb[:, j:j + 1], scale=IWS)
            ju = 12 + mc  # u via fp16 x-projection
            psx = pp4.tile([128, W], F32, tag="ps_g", name="psx")
            mm_bd16(psx[:], xt0, xt1, ju, c0, W, h8_pv)
            gu = pBd.tile([128, W], H, tag="gD_u", bufs=2, name="gu")
            nc.scalar.activation(out=gu[:], in_=psx[:], func=AFT.Tanh,
                                 bias=bb_sb[:, ju:ju + 1], scale=IWS)
            if last:
                cn = pBd.tile([128, W], F32, tag="cnD", bufs=1, name="cn")
                c_dst = cn[:]
            else:
                c_dst = cb[:, mc, c0:c0 + W]
            p0 = c0 // 2
            pc = c_pv[:, mc, p0:p0 + W // 2]
            t1 = pBd.tile([128, W], F32, tag="t1D", bufs=1, name="t1")
            nc.vector.tensor_mul(
                t1[:].rearrange("p (t two) -> p t two", two=2),
                g3[:, 1, :].rearrange("p (t two) -> p t two", two=2),
                _bcast2(pc))
            nc.vector.tensor_mul(c_dst, g3[:, 0, :], gu[:])
            nc.vector.tensor_add(c_dst, c_dst, t1[:])
            th = pBd.tile([128, W], H, tag="thD", bufs=2, name="th")
            nc.scalar.activation(out=th[:], in_=c_dst, func=AFT.Tanh)
            if last:
                hm = pBd.tile([128, W], H, tag="hD", bufs=2, name="hm")
                nc.vector.tensor_mul(hm[:], g3[:, 2, :], th[:])
                chain_update(pBd, hm[:], 2 ** (l - 3), mc=mc)
            else:
                nc.vector.tensor_mul(hb[:, mc, c0:c0 + W], g3[:, 2, :], th[:])
                # parent h stored child-duplicated (stride-0 input view) so
                # the next level's recurrence streams straight into its psum
                nc.vector.tensor_copy(
                    out=h8b[:, mc, 2 * c0:2 * c0 + 2 * W].rearrange(
                        "p (t two) -> p t two", two=2),
                    in_=_bcast2(hb[:, mc, c0:c0 + W]))

        # ---- interleave: A-shallow levels zipped with B-deep l11/l12 ----
        pAs = open_pool("pAsh", bufs=1)
        hA, cA = h11, c11
        h11b, c11b = alloc_hc(hcB, 256)
        h12b, c12b = alloc_hc(hcB, 512)
        h11b8 = hcB.tile([128, KCH, 512], F8, tag="h8b11d", bufs=1,
                         name="h11b8")
        h12b8 = hcB.tile([128, KCH, 1024], F8, tag="h8b12d", bufs=1,
                         name="h12b8")

        xt13 = {}
        xt12 = [None]
        sched = [("A", 10), ("B11", 0), ("A", 9), ("B11", 1), ("A", 8),
                 ("B11", 2), ("A", 7), ("B11", 3), ("A", 6), ("B12", 0),
                 ("A", 5), ("B12", 1), ("A", 4), ("B12", 2), ("A", 3),
                 ("B12", 3)]
        for kind, v in sched:
            if kind == "A":
                hA, cA = phaseA_level(pAs, hcA, 2 ** (v - 3), L_OFF[v],
                                      xgshA, hA, cA, wha_sb, wfh_sb)
            elif kind == "B11":
                if v == 0:
                    bd["xt"] = bdeep_load_xt(11, 0, 256)
                elif v == 1:
                    xt12[0] = bdeep_load_xt(12, 0, 512)
                bdeep_mc(11, 0, v, 256, h11b, h11b8, c11b, bd["h8"],
                         bd["c"])
                if v == 3:
                    chain_update(pBd, h11b[:, :, 0:256], 256)
            else:
                if v == 0:
                    bd["xt"] = xt12[0]
                if v == 1:
                    xt13[0] = bdeep_load_xt(13, 0, 512)
                bdeep_mc(12, 0, v, 512, h12b, h12b8, c12b, h11b8, c11b)
                if v == 3:
                    chain_update(pBd, h12b[:, :, 0:512], 512)

        # stage this core's level-3 (h, c) for the host-side tree top
        nc.vector.tensor_copy(out=stage[:, 0:4], in_=hA[:, :, 0])
        nc.vector.tensor_copy(out=stage[:, 4:8], in_=cA[:, :, 0])

        close_pool("pAsh")
        close_pool("pBatchA")

        # ---- B-deep level 13 (storeless) ----
        for c0 in (0, 512):
            bd["xt"] = xt13[c0]
            for mc in range(4):
                if c0 == 0 and mc == 1:
                    xt13[512] = bdeep_load_xt(13, 512, 512)
                bdeep_mc(13, c0, mc, 512, None, None, None, h12b8, c12b)
        close_pool("pBdeep")
        close_pool("hcA")
        close_pool("pBsh")
        close_pool("hcB")

        # final output: [h3 | c3 | cmax] transposed on the PE so the DRAM
        # write is 12 contiguous 512B descriptors.
        nc.vector.tensor_copy(out=stage[:, 8:12], in_=cmax[:])
        psT = pp4.tile([128, 128], F32, tag="ps_g", name="psT")
        nc.tensor.matmul(psT[0:12, :], stage[:], ident_sb[:],
                         is_transpose=True)
        stageT = pT.tile([128, 128], F32, tag="stageT", bufs=1)
        nc.scalar.activation(out=stageT[0:12, :], in_=psT[0:12, :],
                             func=AFT.Identity)
        nc.sync.dma_start(
            out=out.ap()[0, :].rearrange("(q n) -> q n", q=12),
            in_=stageT[0:12, :])

        close_pool("pTop")
        close_pool("pp2")
        close_pool("pp4")
        close_pool("persist")

    nc.compile()
    return nc


def _host_inputs(inputs, ifoux_w, ifoux_b, ious_w, ious_b, fh_w, fh_b,
                 iofux_w, iofux_b, iofuh_w, iofuh_b):
    """Build the 8 per-core input maps (host-side sharding / layout only)."""
    f32 = np.float32
    inputs = np.asarray(inputs, f32)
    m = MEM

    import ml_dtypes
    E4 = ml_dtypes.float8_e4m3
    f16 = np.float16

    def pk(a):
        """[K, M] -> partition-major [128, (K/128)*M] (fat-DMA layout)."""
        k, mm_ = a.shape
        return np.ascontiguousarray(
            a.reshape(k // 128, 128, mm_).transpose(1, 0, 2).reshape(128, -1))

    def hl8(w):
        wt = np.ascontiguousarray(np.asarray(w, f32).T) * WS
        hi = wt.astype(E4)
        lo = (wt - hi.astype(f32)).astype(E4)
        return hi, lo

    wxa8h, wxa8l_ = hl8(ifoux_w)
    wxa8i = pk(wxa8h[:, 0:m])
    wxa8f = pk(wxa8h[:, m:2 * m])
    wxa8ou = pk(wxa8h[:, 2 * m:4 * m])
    wxa8l = pk(wxa8l_)
    wha = pk(np.asarray(ious_w, f32).T.astype(f16))
    wfhT = pk(np.asarray(fh_w, f32).T.astype(f16))
    wha8 = pk((np.asarray(ious_w, f32).T * WS).astype(E4))
    wfh8 = pk((np.asarray(fh_w, f32).T * WS).astype(E4))
    wxbT = np.asarray(iofux_w, f32).T  # [IN, 4m], blocks i,o,f,u
    wxb8 = pk((np.concatenate([wxbT[:, 0:m], wxbT[:, 2 * m:3 * m],
                               wxbT[:, m:2 * m]], axis=1) * WS).astype(E4))
    wxbou = pk((wxbT[:, 3 * m:4 * m] * WS).astype(f16))
    whb8 = pk((np.asarray(iofuh_w, f32).T * WS).astype(E4))

    ifoux_b = np.asarray(ifoux_b, f32)
    ious_b = np.asarray(ious_b, f32)
    fh_b = np.asarray(fh_b, f32)
    # phase-A folded biases in j-order i,o,u,f
    ba = np.concatenate([
        ifoux_b[0:m] + ious_b[0:m],                  # i
        ifoux_b[2 * m:3 * m] + ious_b[m:2 * m],      # o
        ifoux_b[3 * m:4 * m] + ious_b[2 * m:3 * m],  # u
        ifoux_b[m:2 * m] + fh_b,                     # f (+ fh bias)
    ])
    ba = np.ascontiguousarray(ba.reshape(16, 128).T)
    bb = np.asarray(iofux_b, f32) + np.asarray(iofuh_b, f32)
    bb = np.ascontiguousarray(bb.reshape(16, 128).T)
    bb64 = np.ascontiguousarray(bb * np.float32(WS))
    identity = np.eye(128, dtype=f32)

    in_maps = []
    for k in range(NCORES):
        idx = [np.arange(7)]
        for l in range(3, DEPTH):
            w = 2 ** (l - 3)
            idx.append((2 ** l - 1) + k * w + np.arange(w))
        idx = np.concatenate(idx)
        xf = inputs[idx].T                             # [IN, NCOLS] f32
        xk = xf.astype(np.float16)
        xr = xk.reshape(KCX, 128, NCOLS)               # [kc, p, n]
        xr8 = xf.astype(E4).reshape(KCX, 128, NCOLS)
        x8lo = (xf - xf.astype(E4).astype(f32)).astype(E4)
        xr8l = x8lo.reshape(KCX, 128, NCOLS)
        xp = np.empty((128, XPACK_LEN), np.float16)
        xp8 = np.empty((128, XPACK8_LEN), E4)

        def seg2p(seg):  # [kc', p, w] -> [p, kc'*w]
            kcn, _, w = seg.shape
            return seg.transpose(1, 0, 2).reshape(128, kcn * w)

        xp[:, XO_SH:XO_SH + KCX * SH_COLS] = seg2p(xr[:, :, 0:SH_COLS])
        for (lv, c0, h), off in XO_B.items():
            base = L_OFF[lv] + c0
            w = 256 if lv == 11 else 512
            xp[:, off:off + 4 * w] = seg2p(xr[4 * h:4 * h + 4, :,
                                              base:base + w])
        for (lv, c0), off in XO_A.items():
            base = L_OFF[lv] + c0
            xp8[:, off:off + KCX * 256] = seg2p(xr8[:, :, base:base + 256])
        xsh8h = np.ascontiguousarray(seg2p(xr8[:, :, 0:SH_COLS]))
        xsh8l = np.ascontiguousarray(seg2p(xr8l[:, :, 0:SH_COLS]))
        xp = np.ascontiguousarray(xp)
        xp8 = np.ascontiguousarray(xp8)
        sel = np.zeros((128, 4), f32)
        sel[:, k // 2] = 1.0
        in_maps.append({
            "xpack": xp, "xpack8": xp8, "xsh8h": xsh8h, "xsh8l": xsh8l,
            "wxa8i": wxa8i, "wxa8ou": wxa8ou, "wxa8f": wxa8f,
            "wxa8l": wxa8l, "wha": wha, "wfh": wfhT,
            "wha8": wha8, "wfh8": wfh8, "wxb8": wxb8, "wxbou": wxbou,
            "whb8": whb8, "ba": ba, "bb": bb, "bb64": bb64, "sel3": sel,
            "ident": identity,
        })
    return in_maps


def _host_top(h3, c3, inputs, ifoux_w, ifoux_b, ious_w, ious_b, fh_w, fh_b):
    """fp32 ChildSum over the top 3 levels (nodes 0..6) from the gathered
    level-3 children (nodes 7..14)."""
    f32 = np.float32
    m = MEM

    def sig(v):
        return 1.0 / (1.0 + np.exp(-v))

    xg = (np.asarray(inputs[0:7], f32) @ np.asarray(ifoux_w, f32).T
          + np.asarray(ifoux_b, f32))
    ix, fx = xg[:, :m], xg[:, m:2 * m]
    ox, ux = xg[:, 2 * m:3 * m], xg[:, 3 * m:]
    wi = np.asarray(ious_w, f32)
    bi = np.asarray(ious_b, f32)
    wf = np.asarray(fh_w, f32)
    bf = np.asarray(fh_b, f32)
    h = np.zeros((7, m), f32)
    c = np.zeros((7, m), f32)
    ch_h = np.asarray(h3, f32).reshape(4, 2, m)
    ch_c = np.asarray(c3, f32).reshape(4, 2, m)
    for lvl in (2, 1, 0):
        idx = np.arange(2 ** lvl - 1, 2 ** (lvl + 1) - 1)
        if lvl < 2:
            ch = np.stack([2 * idx + 1, 2 * idx + 2], axis=1)
            ch_h = h[ch]
            ch_c = c[ch]
        hsum = ch_h.sum(axis=1)
        iou = hsum @ wi.T + bi
        i = sig(ix[idx] + iou[:, :m])
        o = sig(ox[idx] + iou[:, m:2 * m])
        u = np.tanh(ux[idx] + iou[:, 2 * m:])
        f = sig(ch_h @ wf.T + bf + fx[idx][:, None, :])
        c[idx] = i * u + (f * ch_c).sum(axis=1)
        h[idx] = o * np.tanh(c[idx])
    return h[0]


def _get_prog():
    global _PROG
    if _PROG is None:
        _PROG = build()
    return _PROG


def kernel(inputs, ifoux_w, ifoux_b, ious_w, ious_b, fh_w, fh_b,
           iofux_w, iofux_b, iofuh_w, iofuh_b, depth=DEPTH, **_unused):
    assert int(depth) == DEPTH, f"kernel hardcodes depth={DEPTH}"
    nc = _get_prog()
    in_maps = _host_inputs(inputs, ifoux_w, ifoux_b, ious_w, ious_b,
                           fh_w, fh_b, iofux_w, iofux_b, iofuh_w, iofuh_b)
    res = run_bass_kernel_spmd(nc, in_maps, list(range(NCORES)))
    outs = [res.results[k]["out"][0] for k in range(NCORES)]
    h3 = np.stack([o[0:MEM] for o in outs])          # nodes 7..14
    c3 = np.stack([o[MEM:2 * MEM] for o in outs])
    cmax = np.max(np.stack([o[2 * MEM:] for o in outs]), axis=0)
    frep = _host_top(h3, c3, inputs, ifoux_w, ifoux_b, ious_w, ious_b,
                     fh_w, fh_b)
    return np.concatenate([frep, cmax])[None, :].astype(np.float32)


if __name__ == "__main__":
    import sys
    if len(sys.argv) > 1 and sys.argv[1] == "emit":
        real_compile = bacc.Bacc.compile
        bacc.Bacc.compile = lambda self: None
        try:
            build()
            print("emit OK")
        finally:
            bacc.Bacc.compile = real_compile


# revision 53
# speedup vs baseline: 1.2074x; 1.2074x over previous
"""BiTreeLSTM (ChildSum bottom-up + Chain top-down) over a complete binary tree,
depth 14 (16383 nodes), on 8 Trainium2 NeuronCores.

Sharding: per-level contiguous node sharding. Core k owns, for every level
l >= 3, the k-th contiguous 1/8 slice of that level's nodes. Children of core
k's nodes at level l are exactly core k's nodes at level l+1, so both
recursions are communication-free: each core returns its level-3 (h, c) plus
its phase-B running max, and the host finishes the 7-node tree top in fp32
(no on-device collective at all).

Compute layout: everything transposed ([feature, node]). Phase-A deep levels
and the phase-B i/f gate x-projections ride naive-fp8 DoubleRow GEMMs (2x PE
rate); phase-B o/u gates stay fp16 (the B max is sensitive to their noise).
The x-projection GEMM and the recurrence GEMM accumulate into the same PSUM
tile, so the big [N, 4*mem] intermediates are never materialized.
"""

import numpy as np

import concourse.bass as bass
import concourse.mybir as mybir
import concourse.tile as tile
from concourse import bacc
from concourse.bass_utils import run_bass_kernel_spmd

AFT = mybir.ActivationFunctionType
ALU = mybir.AluOpType
DR = mybir.MatmulPerfMode.DoubleRow
H = mybir.dt.float16
F32 = mybir.dt.float32
F8 = mybir.dt.float8e4

DEPTH = 14
IN = 1024
MEM = 512
NCORES = 8
KCX = IN // 128   # 8 contraction chunks for x projections
KCH = MEM // 128  # 4 contraction chunks for h projections
WS = 64.0         # host-side scale on fp8 weights (subnormal avoidance)
IWS = float(1.0 / WS)

# per-core column layout: cols 0..6 = global nodes 0..6 (replicated);
# then levels 3..13 contiguously (core-local slices)
L_OFF = {}
_off = 7
for _l in range(3, DEPTH):
    L_OFF[_l] = _off
    _off += 2 ** (_l - 3)
NCOLS = _off          # 2054
SH_COLS = L_OFF[11]   # 262: top7 + levels 3..10

# packed x layout: per-partition contiguous segments so every x DMA is
# 128 fat descriptors. fp16 pack: shallow + B-deep halves; fp8 pack: the
# deep 256-col chunks (consumed as naive-DoubleRow moving pairs by BOTH
# the phase-A deep cells and the phase-B deep i/f gates).
XO_SH = 0
_o = KCX * SH_COLS
XO_B = {}
for _lv, _c, _W in ((11, 0, 256), (12, 0, 512), (13, 0, 512), (13, 512, 512)):
    for _h in (0, 1):
        XO_B[(_lv, _c, _h)] = _o
        _o += (KCX // 2) * _W
XPACK_LEN = _o
XO_A = {}
_o = 0
for _lv, _c in ((13, 0), (13, 256), (13, 512), (13, 768), (12, 0),
                (12, 256), (11, 0)):
    XO_A[(_lv, _c)] = _o
    _o += KCX * 256
XPACK8_LEN = _o

# phase-A gate-chunk order j: i(0..3) o(4..7) u(8..11) f(12..15);
# wxa (ifoux) block layout is i,f,o,u -> block index for each j:
WXA_BLK = [0, 1, 2, 3, 8, 9, 10, 11, 12, 13, 14, 15, 4, 5, 6, 7]
# phase-B j order i(0..3) o(4..7) f(8..11) u(12..15).
# wxb8 packs i,f,o blocks (fp8): j -> block; wxbou packs u blocks (fp16 x64).
BXB8 = {j: p for p, j in enumerate([0, 1, 2, 3, 8, 9, 10, 11, 4, 5, 6, 7])}
BXB16 = {j: p for p, j in enumerate([12, 13, 14, 15])}

_PROG = None


def _bcast2(ap):
    """View [P, ..., N] as [P, ..., N, 2] with step 0 (each element twice)."""
    return bass.AP(tensor=ap.tensor, offset=ap.offset, ap=ap.ap + [[0, 2]])


def _dup_mid(ap):
    """View [P, ..., X] as [P, ..., 2, X] with step 0 on the new dim."""
    return bass.AP(tensor=ap.tensor, offset=ap.offset,
                   ap=ap.ap[:-1] + [[0, 2]] + [ap.ap[-1]])


def _pairs(ap):
    """(even, odd) views of the last dim interpreted as [..., t, 2]."""
    nd = len(ap.shape)
    letters = [chr(ord("a") + i) for i in range(nd - 1)]
    spec = " ".join(letters) + " (t two) -> " + " ".join(letters) + " t two"
    v = ap.rearrange(spec, two=2)
    idx = (slice(None),) * nd
    return v[idx + (0,)], v[idx + (1,)]


def build():
    """Build + compile the SPMD Bass program. Returns the Bacc object."""
    nc = bacc.Bacc("TRN2", target_bir_lowering=False, debug=False,
                   num_devices=NCORES)

    xpack = nc.dram_tensor("xpack", [128, XPACK_LEN], H,
                           kind="ExternalInput")
    xpack8 = nc.dram_tensor("xpack8", [128, XPACK8_LEN], F8,
                            kind="ExternalInput")
    xsh8h = nc.dram_tensor("xsh8h", [128, KCX * SH_COLS], F8,
                           kind="ExternalInput")
    xsh8l = nc.dram_tensor("xsh8l", [128, KCX * SH_COLS], F8,
                           kind="ExternalInput")
    # weights are host-packed partition-major ([128, kc*cols]) so every
    # load is a fat contiguous DMA -- strided-view loads cost 2-4us of
    # trigger time each on the issuing engine queue
    wxa8i = nc.dram_tensor("wxa8i", [128, KCX * MEM], F8,
                           kind="ExternalInput")
    wxa8ou = nc.dram_tensor("wxa8ou", [128, KCX * 2 * MEM], F8,
                            kind="ExternalInput")
    wxa8f = nc.dram_tensor("wxa8f", [128, KCX * MEM], F8,
                           kind="ExternalInput")
    wxa8l = nc.dram_tensor("wxa8l", [128, KCX * 4 * MEM], F8,
                           kind="ExternalInput")
    wha8 = nc.dram_tensor("wha8", [128, KCH * 3 * MEM], F8,
                          kind="ExternalInput")
    wfh8 = nc.dram_tensor("wfh8", [128, KCH * MEM], F8,
                          kind="ExternalInput")
    wha = nc.dram_tensor("wha", [128, KCH * 3 * MEM], H,
                         kind="ExternalInput")
    wfh = nc.dram_tensor("wfh", [128, KCH * MEM], H, kind="ExternalInput")
    wxb8 = nc.dram_tensor("wxb8", [128, KCX * 3 * MEM], F8,
                          kind="ExternalInput")
    wxbou = nc.dram_tensor("wxbou", [128, KCX * MEM], H,
                           kind="ExternalInput")
    whb8 = nc.dram_tensor("whb8", [128, KCH * 4 * MEM], F8,
                          kind="ExternalInput")
    ba = nc.dram_tensor("ba", [128, 16], F32, kind="ExternalInput")
    bb = nc.dram_tensor("bb", [128, 16], F32, kind="ExternalInput")
    bb64 = nc.dram_tensor("bb64", [128, 16], F32, kind="ExternalInput")
    sel3 = nc.dram_tensor("sel3", [128, 4], F32, kind="ExternalInput")
    ident = nc.dram_tensor("ident", [128, 128], F32, kind="ExternalInput")
    # out: [h3 | c3 | cmax] per core, host finishes the tree top
    out = nc.dram_tensor("out", [1, 3 * MEM], F32, kind="ExternalOutput")

    def xp_view(off, ln, kc):
        return xpack.ap()[:, off:off + ln].rearrange("p (kc n) -> p kc n",
                                                     kc=kc)

    pool_stack = []

    with tile.TileContext(nc) as tc:

        def open_pool(name, bufs=1, space="SBUF"):
            cm = tc.tile_pool(name=name, bufs=bufs, space=space)
            p = cm.__enter__()
            pool_stack.append((name, cm))
            return p

        def close_pool(name):
            n, cm = pool_stack.pop()
            assert n == name, f"pool close order: expected {n}, got {name}"
            cm.__exit__(None, None, None)

        persist = open_pool("persist")
        pp4 = open_pool("pp4", bufs=6, space="PSUM")
        pp2 = open_pool("pp2", bufs=2, space="PSUM")

        ba_sb = persist.tile([128, 16], F32, tag="ba")
        bb_sb = persist.tile([128, 16], F32, tag="bb")
        bb64_sb = persist.tile([128, 16], F32, tag="bb64")
        sel3_sb = persist.tile([128, 4], F32, tag="sel3")
        ident_sb = persist.tile([128, 128], F32, tag="ident")
        cmax = persist.tile([128, 4], F32, tag="cmax")
        stage = persist.tile([128, 12], F32, tag="stage")
        wxa8_t = persist.tile([128, KCX, 2, 4 * MEM], F8, tag="wxa8",
                              name="wxa8")
        wxb8_t = persist.tile([128, KCX, 3 * MEM], F8, tag="wxb8",
                              name="wxb8")
        wxbou_t = [persist.tile([128, MEM], H, tag=f"wxbou{kc}",
                                name=f"wxbou{kc}") for kc in range(KCX)]
        wha_sb = persist.tile([128, KCH, 3 * MEM], H, tag="wha")
        wfh_sb = persist.tile([128, KCH, MEM], H, tag="wfh")
        whb_sb = persist.tile([128, KCH, 4 * MEM], F8, tag="whb")
        xTsh = persist.tile([128, KCX, SH_COLS], H, tag="xTsh")
        xTsh8 = persist.tile([128, KCX, 2, SH_COLS], F8, tag="xTsh8",
                             name="xTsh8")
        xgshB = persist.tile([128, 16, SH_COLS], H, tag="xgshB")

        nc.gpsimd.dma_start(out=ba_sb[:], in_=ba.ap())
        nc.gpsimd.dma_start(out=bb64_sb[:], in_=bb64.ap())
        nc.gpsimd.dma_start(out=bb_sb[:], in_=bb.ap())
        nc.vector.memset(cmax[:], -3.0e38)

        # ============ helpers ============

        def mm_xa8(ps, xt8, blk, start=True, stop=True):
            """deep-A x-projection: naive fp8 DoubleRow over kc pairs."""
            c0, c1 = blk * 128, (blk + 1) * 128
            for q0 in range(0, KCX, 2):
                nc.tensor.matmul(ps, wxa8_t[:, q0:q0 + 2, 0, c0:c1],
                                 xt8[:, q0:q0 + 2, :],
                                 start=(start and q0 == 0),
                                 stop=(stop and q0 + 2 >= KCX), perf_mode=DR)

        def mm_xa15(ps, xt8, blk, start=True, stop=True):
            """shallow-A x-projection: 1.5-term compensated fp8."""
            c0, c1 = blk * 128, (blk + 1) * 128
            for q in range(KCX):
                nc.tensor.matmul(ps, wxa8_t[:, q, :, c0:c1],
                                 _dup_mid(xt8[:, q, 0, :]),
                                 start=(start and q == 0), stop=False,
                                 perf_mode=DR)
            for q0 in range(0, KCX, 2):
                nc.tensor.matmul(ps, wxa8_t[:, q0:q0 + 2, 0, c0:c1],
                                 xt8[:, q0:q0 + 2, 1, :], start=False,
                                 stop=(stop and q0 + 2 >= KCX), perf_mode=DR)

        def mm_ha8(ps, w8, h8, blk, sl, start=True, stop=True):
            """deep-A recurrence: naive fp8 DoubleRow over kc pairs."""
            c0, c1 = blk * 128, (blk + 1) * 128
            for q0 in range(0, KCH, 2):
                nc.tensor.matmul(ps, w8[:, q0:q0 + 2, c0:c1],
                                 h8[:, q0:q0 + 2, sl],
                                 start=(start and q0 == 0),
                                 stop=(stop and q0 + 2 >= KCH), perf_mode=DR)

        def mm_h8(ps, h8, blk, sl, start=True, stop=True):
            """chain recurrence on whb8. DoubleRow for wide moving dims;
            plain per-chunk matmuls below FD=64 (DR's LDWEIGHTS overhead
            loses there)."""
            c0, c1 = blk * 128, (blk + 1) * 128
            fd = sl.stop - sl.start
            if fd >= 64:
                for q0 in range(0, KCH, 2):
                    nc.tensor.matmul(ps, whb_sb[:, q0:q0 + 2, c0:c1],
                                     h8[:, q0:q0 + 2, sl],
                                     start=(start and q0 == 0),
                                     stop=(stop and q0 + 2 >= KCH),
                                     perf_mode=DR)
            else:
                for q in range(KCH):
                    nc.tensor.matmul(ps, whb_sb[:, q, c0:c1], h8[:, q, sl],
                                     start=(start and q == 0),
                                     stop=(stop and q == KCH - 1))

        def mm_bd8(ps, segs, j, c0, h8d):
            """phase-B deep i/f/o gate psum: per 256-col half, the naive-fp8
            x-projection opens the accumulation and that half's recurrence
            GEMM (parent-duplicated fp8 h) closes it."""
            blk = BXB8[j]
            x0, x1 = blk * 128, (blk + 1) * 128
            r0, r1 = j * 128, (j + 1) * 128
            for s, seg in enumerate(segs):
                o = s * 256
                for q0 in range(0, KCX, 2):
                    nc.tensor.matmul(ps[:, o:o + 256],
                                     wxb8_t[:, q0:q0 + 2, x0:x1],
                                     seg[:, q0:q0 + 2, :],
                                     start=q0 == 0, stop=False,
                                     perf_mode=DR)
                for q0 in range(0, KCH, 2):
                    nc.tensor.matmul(ps[:, o:o + 256],
                                     whb_sb[:, q0:q0 + 2, r0:r1],
                                     h8d[:, q0:q0 + 2, c0 + o:c0 + o + 256],
                                     start=False, stop=q0 + 2 >= KCH,
                                     perf_mode=DR)

        def mm_bd16(ps, xt0, xt1, j, c0, W, h8d):
            """phase-B deep u gate psum: fp16 x-projection + fp8 recurrence,
            per 256-col half as in mm_bd8."""
            blk = BXB16[j]
            x0, x1 = blk * 128, (blk + 1) * 128
            r0, r1 = j * 128, (j + 1) * 128
            for o in range(0, W, 256):
                for kc in range(KCX):
                    xt_sl = (xt0[:, kc, o:o + 256] if kc < KCX // 2
                             else xt1[:, kc - KCX // 2, o:o + 256])
                    nc.tensor.matmul(ps[:, o:o + 256],
                                     wxbou_t[kc][:, x0:x1], xt_sl,
                                     start=kc == 0, stop=False)
                for q0 in range(0, KCH, 2):
                    nc.tensor.matmul(ps[:, o:o + 256],
                                     whb_sb[:, q0:q0 + 2, r0:r1],
                                     h8d[:, q0:q0 + 2, c0 + o:c0 + o + 256],
                                     start=False, stop=q0 + 2 >= KCH,
                                     perf_mode=DR)

        def alloc_hc(pool, M, with_c=True):
            Mp = max(M, 2)  # matmul moving dim must be >= 2; pad tiny levels
            h = pool.tile([128, KCH, Mp], H, tag=f"h{M}", bufs=1,
                          name=f"h{M}")
            c = (pool.tile([128, KCH, Mp], F32, tag=f"c{M}", bufs=1,
                           name=f"c{M}") if with_c else None)
            # pad columns are never read (their matmul psum outputs are
            # never consumed), so they stay uninitialized
            return h, c

        def csum_cell(pool, W, xt8, hs8, chh8, c_pv, cb, h_dst,
                      c_dst, d0, wha8_t, wfh8_t, leaf=False):
            """Deep child-sum cell, all-fp8 naive DoubleRow GEMMs (64x psums).

            xt8: [128, KCX, W] fp8 x chunk; hs8: [128, KCH, W] fp8 child
            h-sum; chh8: [128, KCH, 2W] fp8 interleaved children h."""
            for mc in range(4):
                gates = {}
                for gi, j in (("i", mc), ("o", 4 + mc), ("u", 8 + mc)):
                    ps = pp4.tile([128, W], F32, tag="ps_g", name="psg")
                    mm_xa8(ps[:], xt8, WXA_BLK[j], start=True, stop=leaf)
                    if not leaf:
                        mm_ha8(ps[:], wha8_t, hs8, j, slice(0, W),
                               start=False, stop=True)
                    g = pool.tile([128, W], H, tag=f"g_{gi}", name="g")
                    nc.scalar.activation(
                        out=g[:], in_=ps[:],
                        func=AFT.Tanh if gi == "u" else AFT.Sigmoid,
                        bias=ba_sb[:, j:j + 1], scale=IWS)
                    gates[gi] = g
                c_sl = c_dst[:, mc, d0:d0 + W]
                h_sl = h_dst[:, mc, d0:d0 + W]
                if leaf:
                    nc.vector.tensor_mul(c_sl, gates["i"][:], gates["u"][:])
                else:
                    psx = pp4.tile([128, W], F32, tag="ps_g", name="psx")
                    mm_xa8(psx[:], xt8, WXA_BLK[12 + mc])
                    fx = pool.tile([128, W], F32, tag="fx", bufs=1, name="fx")
                    nc.scalar.activation(out=fx[:], in_=psx[:],
                                         func=AFT.Identity,
                                         bias=ba_sb[:, 12 + mc:13 + mc],
                                         scale=IWS)
                    psF = pp2.tile([128, 2 * W], F32, tag="ps_F", name="psF")
                    mm_ha8(psF[:], wfh8_t, chh8, mc, slice(0, 2 * W))
                    f_sb = pool.tile([128, 2 * W], H, tag="f", bufs=1,
                                     name="f")
                    nc.vector.scalar_tensor_tensor(
                        f_sb[:].rearrange("p (t two) -> p t two", two=2),
                        psF[:].rearrange("p (t two) -> p t two", two=2),
                        IWS, _bcast2(fx[:]), op0=ALU.mult, op1=ALU.add)
                    nc.scalar.activation(out=f_sb[:], in_=f_sb[:],
                                         func=AFT.Sigmoid)
                    nc.vector.tensor_mul(c_sl, gates["i"][:], gates["u"][:])
                    f_e, f_o = _pairs(f_sb[:])
                    c_e, c_o = _pairs(c_pv[:, mc, cb:cb + 2 * W])
                    t1 = pool.tile([128, W], H, tag="t1", bufs=1, name="t1")
                    nc.vector.tensor_mul(t1[:], f_e, c_e)
                    nc.vector.tensor_add(c_sl, c_sl, t1[:])
                    t2 = pool.tile([128, W], H, tag="t2", bufs=1, name="t2")
                    nc.vector.tensor_mul(t2[:], f_o, c_o)
                    nc.vector.tensor_add(c_sl, c_sl, t2[:])
                th = pool.tile([128, W], H, tag="th", bufs=1, name="th")
                nc.scalar.activation(out=th[:], in_=c_sl, func=AFT.Tanh)
                nc.vector.tensor_mul(h_sl, gates["o"][:], th[:])

        def phaseA_level(pool, hc_pool, M, xg_off, xg_t, h_pv, c_pv,
                         wha_t, wfh_t, ppg=None, ppF=None,
                         gtag="ps_g"):
            """Shallow child-sum level from precomputed x pre-activations."""
            ppg = ppg or pp4
            ppF = ppF or pp2
            h_cur, c_cur = alloc_hc(hc_pool, M)
            Mp = max(M, 2)
            hs = pool.tile([128, KCH, Mp], H, tag="hsumS", bufs=1, name="hs")
            he, ho = _pairs(h_pv[:, :, 0:2 * M])
            nc.vector.tensor_add(hs[:, :, 0:M], he, ho)
            gate = pool.tile([128, 12, M], H, tag="gateA", bufs=1,
                             name="gate")
            gsz = max(1, 512 // Mp)
            for g0 in range(0, 12, gsz):
                g1 = min(12, g0 + gsz)
                ps = ppg.tile([128, g1 - g0, Mp], F32, tag=gtag, name="psg")
                for j in range(g0, g1):
                    for kc in range(KCH):
                        nc.tensor.matmul(
                            ps[:, j - g0, :],
                            wha_t[:, kc, j * 128:(j + 1) * 128],
                            hs[:, kc, :], start=kc == 0, stop=kc == KCH - 1)
                pre = pool.tile([128, g1 - g0, M], F32, tag="preA", name="pre")
                nc.vector.tensor_add(pre[:], xg_t[:, g0:g1, xg_off:xg_off + M],
                                     ps[:, :, 0:M])
                if g0 < 8:
                    s1 = min(g1, 8)
                    nc.scalar.activation(out=gate[:, g0:s1, :],
                                         in_=pre[:, 0:s1 - g0, :],
                                         func=AFT.Sigmoid)
                if g1 > 8:
                    s0 = max(g0, 8)
                    nc.scalar.activation(out=gate[:, s0:g1, :],
                                         in_=pre[:, s0 - g0:g1 - g0, :],
                                         func=AFT.Tanh)
            fga = pool.tile([128, 4, 2 * M], H, tag="fgA", bufs=1,
                            name="fga")
            gF = max(1, 512 // (2 * M))
            for m0 in range(0, 4, gF):
                m1 = min(4, m0 + gF)
                psF = ppF.tile([128, m1 - m0, 2 * M], F32, tag="ps_F",
                               name="psF")
                for mc in range(m0, m1):
                    for kc in range(KCH):
                        nc.tensor.matmul(
                            psF[:, mc - m0, :],
                            wfh_t[:, kc, mc * 128:(mc + 1) * 128],
                            h_pv[:, kc, 0:2 * M], start=kc == 0,
                            stop=kc == KCH - 1)
                fxv = xg_t[:, 12 + m0:12 + m1, xg_off:xg_off + M]
                nc.vector.tensor_add(
                    fga[:, m0:m1, :].rearrange("p m (t two) -> p m t two",
                                               two=2),
                    psF[:].rearrange("p m (t two) -> p m t two", two=2),
                    _bcast2(fxv))
            nc.scalar.activation(out=fga[:], in_=fga[:], func=AFT.Sigmoid)
            fe, fo = _pairs(fga[:])
            ce, co = _pairs(c_pv[:, :, 0:2 * M])
            c_sl = c_cur[:, :, 0:M]
            h_sl = h_cur[:, :, 0:M]
            nc.vector.tensor_mul(c_sl, gate[:, 0:4, :], gate[:, 8:12, :])
            t1 = pool.tile([128, 4, M], H, tag="t1A", bufs=1, name="t1")
            nc.vector.tensor_mul(t1[:], fe, ce)
            nc.vector.tensor_add(c_sl, c_sl, t1[:])
            t2 = pool.tile([128, 4, M], H, tag="t2A", bufs=1, name="t2")
            nc.vector.tensor_mul(t2[:], fo, co)
            nc.vector.tensor_add(c_sl, c_sl, t2[:])
            th = pool.tile([128, 4, M], H, tag="thA", bufs=1, name="th")
            nc.scalar.activation(out=th[:], in_=c_sl, func=AFT.Tanh)
            nc.vector.tensor_mul(h_sl, gate[:, 4:8, :], th[:])
            return h_cur, c_cur

        # static pool for the final-output transpose
        pT = open_pool("pTop", bufs=1)

        # ============ phase A ============

        def chain_update(pool, h_ap, M, mc=None, eng=None):
            eng = eng or nc.vector
            if mc is not None:  # single mem-chunk slab [128, M]
                rm = pool.tile([128, 1], F32, tag="rmax", bufs=1, name="rm")
                eng.tensor_reduce(out=rm[:], in_=h_ap,
                                  axis=mybir.AxisListType.X, op=ALU.max)
                eng.tensor_max(cmax[:, mc:mc + 1], cmax[:, mc:mc + 1],
                               rm[:])
            elif M == 1:
                rm1 = pool.tile([128, KCH], F32, tag="rmaxq", bufs=1,
                                name="rm1")
                eng.tensor_copy(out=rm1[:], in_=h_ap[:, :, 0])
                eng.tensor_max(cmax[:], cmax[:], rm1[:])
            else:
                rm = pool.tile([128, KCH], F32, tag="rmaxq", bufs=1, name="rm")
                eng.tensor_reduce(out=rm[:], in_=h_ap,
                                  axis=mybir.AxisListType.X, op=ALU.max)
                eng.tensor_max(cmax[:], cmax[:], rm[:])

        def phaseB_level(pool, M, xg_off, h8_pv, c_pv, direct=False,
                         root=False, dup=False, need_h8=True):
            """Chain level. h8_pv: fp8 parent h [128, KCH, Pp].
            Returns (h_cur, c_cur, h8_cur); h8_cur is child-duplicated
            [128, KCH, 2M] when dup=True (feeds the B-deep recurrence)."""
            h_cur, c_cur = alloc_hc(hcB, M)
            Mp = max(M, 2)
            h8_cur = None
            if need_h8:
                h8_cur = hcB.tile([128, KCH, (2 if dup else 1) * Mp], F8,
                                  tag=f"h8b{M}{'d' if dup else ''}", bufs=1,
                                  name=f"h8b{M}")
            Pn = M if direct else M // 2
            Pp = max(Pn, 2)
            gate = pool.tile([128, 16, M], H, tag="gateB", bufs=1,
                             name="gate")
            if root:
                nc.scalar.activation(out=gate[:, 0:12, :],
                                     in_=xgshB[:, 0:12, xg_off:xg_off + M],
                                     func=AFT.Sigmoid, scale=IWS)
                nc.scalar.activation(out=gate[:, 12:16, :],
                                     in_=xgshB[:, 12:16, xg_off:xg_off + M],
                                     func=AFT.Tanh, scale=IWS)
            else:
                pre = pool.tile([128, 16, M], H, tag="preB", bufs=1,
                                name="pre")
                gsz = max(1, 512 // Pp)
                for g0 in range(0, 16, gsz):
                    g1 = min(16, g0 + gsz)
                    ps = pp4.tile([128, g1 - g0, Pp], F32, tag="ps_g",
                                  name="psg")
                    for j in range(g0, g1):
                        mm_h8(ps[:, j - g0, :], h8_pv, j, slice(0, Pp))
                    xgv = xgshB[:, g0:g1, xg_off:xg_off + M]
                    psv = ps[:, :, 0:Pn]
                    if direct:
                        nc.vector.tensor_add(pre[:, g0:g1, :], xgv, psv)
                    else:
                        nc.vector.tensor_add(
                            pre[:, g0:g1, :].rearrange(
                                "p q (t two) -> p q t two", two=2),
                            xgv.rearrange("p q (t two) -> p q t two", two=2),
                            _bcast2(psv))
                nc.scalar.activation(out=gate[:, 0:12, :], in_=pre[:, 0:12, :],
                                     func=AFT.Sigmoid, scale=IWS)
                nc.scalar.activation(out=gate[:, 12:16, :],
                                     in_=pre[:, 12:16, :], func=AFT.Tanh,
                                     scale=IWS)
            c_sl = c_cur[:, :, 0:M]
            h_sl = h_cur[:, :, 0:M]
            nc.vector.tensor_mul(c_sl, gate[:, 0:4, :], gate[:, 12:16, :])
            if not root:
                if direct:
                    t1 = pool.tile([128, 4, M], H, tag="t1B", bufs=1,
                                   name="t1")
                    nc.vector.tensor_mul(t1[:], gate[:, 8:12, :],
                                         c_pv[:, :, 0:Pn])
                    nc.vector.tensor_add(c_sl, c_sl, t1[:])
                else:
                    fe, fo = _pairs(gate[:, 8:12, :])
                    ce, co = _pairs(c_sl)
                    t1 = pool.tile([128, 4, Pn], H, tag="t1B", bufs=1,
                                   name="t1")
                    nc.vector.tensor_mul(t1[:], fe, c_pv[:, :, 0:Pn])
                    nc.vector.tensor_add(ce, ce, t1[:])
                    t2 = pool.tile([128, 4, Pn], H, tag="t2B", bufs=1,
                                   name="t2")
                    nc.vector.tensor_mul(t2[:], fo, c_pv[:, :, 0:Pn])
                    nc.vector.tensor_add(co, co, t2[:])
            th = pool.tile([128, 4, M], H, tag="thB", bufs=1, name="th")
            nc.scalar.activation(out=th[:], in_=c_sl, func=AFT.Tanh)
            nc.vector.tensor_mul(h_sl, gate[:, 4:8, :], th[:])
            if dup:
                nc.vector.tensor_copy(
                    out=h8_cur[:, :, 0:2 * M].rearrange(
                        "p q (t two) -> p q t two", two=2),
                    in_=_bcast2(h_sl))
            elif need_h8:
                nc.vector.tensor_copy(out=h8_cur[:, :, 0:M], in_=h_sl)
            chain_update(pool, h_cur[:, :, 0:M], M)
            return h_cur, c_cur, h8_cur

        def emit_b3_parent(h_pv, c_pv):
            hpar = pBs.tile([128, KCH, 2], H, tag="hpar", bufs=1)
            hpar8 = pBs.tile([128, KCH, 2], F8, tag="hpar8", bufs=1)
            cpar = pBs.tile([128, KCH, 2], F32, tag="cpar", bufs=1)
            for c in range(4):
                if c == 0:
                    nc.vector.tensor_scalar_mul(hpar[:, :, 0:1],
                                                h_pv[:, :, c:c + 1],
                                                sel3_sb[:, c:c + 1])
                    nc.vector.tensor_scalar_mul(cpar[:, :, 0:1],
                                                c_pv[:, :, c:c + 1],
                                                sel3_sb[:, c:c + 1])
                else:
                    nc.vector.scalar_tensor_tensor(
                        hpar[:, :, 0:1], h_pv[:, :, c:c + 1],
                        sel3_sb[:, c:c + 1], hpar[:, :, 0:1],
                        op0=ALU.mult, op1=ALU.add)
                    nc.vector.scalar_tensor_tensor(
                        cpar[:, :, 0:1], c_pv[:, :, c:c + 1],
                        sel3_sb[:, c:c + 1], cpar[:, :, 0:1],
                        op0=ALU.mult, op1=ALU.add)
            nc.vector.tensor_copy(out=hpar8[:, :, 0:1], in_=hpar[:, :, 0:1])
            return hpar8, cpar

        hcB = open_pool("hcB")
        pBs = open_pool("pBsh", bufs=2)
        hcA = open_pool("hcA")
        hcA12 = open_pool("hcA12")
        wha8_sb = hcA12.tile([128, KCH, 3 * MEM], F8, tag="wha8")
        wfh8_sb = hcA12.tile([128, KCH, MEM], F8, tag="wfh8")
        pAd = open_pool("pAdeep", bufs=2)

        # DMA order: the first leaf cell's operands (xt chunks + wxa8h
        # i/o/u blocks, ~2.3MB) go first on sync so the PE starts ASAP;
        # the B-shallow xg batch operands (xTsh/wxb8/xsh8-hi/wxbou) stream
        # on scalar meanwhile.
        def xp8_view(off, ln):
            return xpack8.ap()[:, off:off + ln].rearrange(
                "p (kc n) -> p kc n", kc=KCX)

        # NO DMA triggers ride the scalar queue: gate activations live
        # there and a ring-credit-throttled trigger backlog delays them.
        # sync = leaf-critical loads; gpsimd (otherwise idle) = the rest.
        xt_pre = []
        for g in range(2):
            xt = pAd.tile([128, KCX, 256], F8, tag="xt", name="xt",
                          bufs=4)
            nc.sync.dma_start(out=xt[:],
                              in_=xp8_view(XO_A[(13, g * 256)], KCX * 256))
            xt_pre.append(xt)
        nc.sync.dma_start(
            out=wxa8_t[:, :, 0, 0:MEM],
            in_=wxa8i.ap().rearrange("p (kc m) -> p kc m", kc=KCX))
        nc.sync.dma_start(
            out=wxa8_t[:, :, 0, 2 * MEM:4 * MEM],
            in_=wxa8ou.ap().rearrange("p (kc m) -> p kc m", kc=KCX))
        nc.gpsimd.dma_start(
            out=xTsh8[:, :, 0, :],
            in_=xsh8h.ap().rearrange("p (kc n) -> p kc n", kc=KCX))
        nc.gpsimd.dma_start(
            out=wxb8_t[:],
            in_=wxb8.ap().rearrange("p (kc m) -> p kc m", kc=KCX))
        nc.gpsimd.dma_start(out=xTsh[:],
                            in_=xp_view(XO_SH, KCX * SH_COLS, KCX))
        for kc in range(KCX):
            [nc.gpsimd, nc.sync][kc % 2].dma_start(
                out=wxbou_t[kc][:],
                in_=wxbou.ap()[:, kc * MEM:(kc + 1) * MEM])
        nc.sync.dma_start(
            out=wxa8_t[:, :, 0, MEM:2 * MEM],
            in_=wxa8f.ap().rearrange("p (kc m) -> p kc m", kc=KCX))
        nc.sync.dma_start(
            out=wha8_sb[:],
            in_=wha8.ap().rearrange("p (kc m) -> p kc m", kc=KCH))
        nc.sync.dma_start(
            out=wfh8_sb[:],
            in_=wfh8.ap().rearrange("p (kc m) -> p kc m", kc=KCH))
        nc.gpsimd.dma_start(
            out=whb_sb[:],
            in_=whb8.ap().rearrange("p (kc m) -> p kc m", kc=KCH))
        nc.gpsimd.dma_start(
            out=wha_sb[:],
            in_=wha.ap().rearrange("p (kc m) -> p kc m", kc=KCH))
        nc.gpsimd.dma_start(
            out=wfh_sb[:],
            in_=wfh.ap().rearrange("p (kc m) -> p kc m", kc=KCH))
        nc.gpsimd.dma_start(
            out=xTsh8[:, :, 1, :],
            in_=xsh8l.ap().rearrange("p (kc n) -> p kc n", kc=KCX))
        nc.gpsimd.dma_start(
            out=wxa8_t[:, :, 1, :],
            in_=wxa8l.ap().rearrange("p (kc m) -> p kc m", kc=KCX))
        nc.gpsimd.dma_start(out=sel3_sb[:], in_=sel3.ap())
        nc.gpsimd.dma_start(out=ident_sb[:], in_=ident.ap())

        # phase-B xg batch: i/f/o blocks via naive fp8 DR; u via fp16 (x64
        # weights) -- every psum lands directly in xgshB's 64x convention.
        def emit_bxg_fp8():
            for j in (0, 1, 2, 3, 8, 9, 10, 11, 4, 5, 6, 7):
                ps = pp2.tile([128, SH_COLS], F32, tag="ps_F", name="psb")
                blk = BXB8[j]
                for q0 in range(0, KCX, 2):
                    nc.tensor.matmul(ps[:],
                                     wxb8_t[:, q0:q0 + 2,
                                            blk * 128:(blk + 1) * 128],
                                     xTsh8[:, q0:q0 + 2, 0, :],
                                     start=q0 == 0, stop=q0 + 2 >= KCX,
                                     perf_mode=DR)
                nc.scalar.activation(out=xgshB[:, j, :], in_=ps[:],
                                     func=AFT.Identity,
                                     bias=bb64_sb[:, j:j + 1])

        def emit_bxg_u():
            for j in (12, 13, 14, 15):
                ps = pp2.tile([128, SH_COLS], F32, tag="ps_F", name="psb")
                blk = BXB16[j]
                for kc in range(KCX):
                    nc.tensor.matmul(ps[:],
                                     wxbou_t[kc][:, blk * 128:(blk + 1) * 128],
                                     xTsh[:, kc, :], start=kc == 0,
                                     stop=kc == KCX - 1)
                nc.scalar.activation(out=xgshB[:, j, :], in_=ps[:],
                                     func=AFT.Identity,
                                     bias=bb64_sb[:, j:j + 1])

        # B-chain emitter: one item per call, sprinkled between A-deep units
        b_state = {"h": None, "c": None, "h8": None, "idx": 0}

        def emit_b_item():
            i = b_state["idx"]
            b_state["idx"] += 1
            if i > 10:
                return
            if i == 0:
                (b_state["h"], b_state["c"],
                 b_state["h8"]) = phaseB_level(pBs, 1, 0, None, None,
                                               root=True)
            elif i in (1, 2):
                m, xo = (2, 1) if i == 1 else (4, 3)
                (b_state["h"], b_state["c"],
                 b_state["h8"]) = phaseB_level(pBs, m, xo, b_state["h8"],
                                               b_state["c"],
                                               need_h8=(i == 1))
            elif i == 3:
                hpar8, cpar = emit_b3_parent(b_state["h"], b_state["c"])
                (b_state["h"], b_state["c"],
                 b_state["h8"]) = phaseB_level(pBs, 1, L_OFF[3], hpar8, cpar,
                                               direct=True)
            else:
                (b_state["h"], b_state["c"],
                 b_state["h8"]) = phaseB_level(pBs, 2 ** (i - 3), L_OFF[i],
                                               b_state["h8"], b_state["c"],
                                               dup=(i == 10))

        def make_hs8_chh8(h_pv, W2):
            """fp8 child-sum + interleaved children from fp16 h_pv."""
            hs8 = pAd.tile([128, KCH, W2 // 2], F8, tag="hs8d", bufs=1,
                           name="hs8")
            he, ho = _pairs(h_pv[:, :, 0:W2])
            nc.vector.tensor_add(hs8[:], he, ho)
            chh8 = pAd.tile([128, KCH, W2], F8, tag="chh8d", bufs=1,
                            name="chh8")
            nc.vector.tensor_copy(out=chh8[:], in_=h_pv[:, :, 0:W2])
            return hs8, chh8

        # levels 13+12 fused: leaf chunks are consumed immediately; the fp8
        # casts for the level-12 recurrence are emitted per leaf chunk so
        # they overlap the next chunk's compute. The B xg batch is emitted
        # after the first leaf cell (its 6MB of operands stream while the
        # PE chews the ~2MB leaf working set).
        def load_xt_a(lv, base):
            xt = pAd.tile([128, KCX, 256], F8, tag="xt", bufs=4)
            nc.sync.dma_start(out=xt[:],
                              in_=xp8_view(XO_A[(lv, base)], KCX * 256))
            return xt

        h12, c12 = alloc_hc(hcA12, 512)
        xt_q = {}
        for c0 in (0, 256):
            h13c = pAd.tile([128, KCH, 512], H, tag="h13c", bufs=1)
            c13c = pAd.tile([128, KCH, 512], F32, tag="c13c", bufs=1)
            hs8 = pAd.tile([128, KCH, 256], F8, tag="hs8d", bufs=1,
                           name="hs8")
            chh8 = pAd.tile([128, KCH, 512], F8, tag="chh8d", bufs=1,
                            name="chh8")
            # prefetch this slab's l12 chunk ahead of the leaf compute so
            # the gpsimd DMA queue never gates the level-12 cell
            xt_q[(12, c0)] = load_xt_a(12, c0)
            for sc in (0, 1):
                base = 2 * c0 + sc * 256
                if c0 == 0:
                    xt = xt_pre[sc]
                else:
                    xt = xt_q[(13, base)]
                csum_cell(pAd, 256, xt, None, None, None, 0,
                          h13c, c13c, sc * 256, wha8_sb, wfh8_sb, leaf=True)
                sl = slice(sc * 256, sc * 256 + 256)
                he, ho = _pairs(h13c[:, :, sl])
                nc.vector.tensor_add(hs8[:, :, sc * 128:sc * 128 + 128],
                                     he, ho)
                nc.vector.tensor_copy(out=chh8[:, :, sl],
                                      in_=h13c[:, :, sl])
                if c0 == 0 and sc == 0:
                    emit_bxg_fp8()
                    xt_q[(13, 512)] = load_xt_a(13, 512)
                    xt_q[(13, 768)] = load_xt_a(13, 768)
                elif c0 == 0 and sc == 1:
                    emit_bxg_u()
                else:
                    emit_b_item()
            csum_cell(pAd, 256, xt_q[(12, c0)], hs8, chh8, c13c, 0,
                      h12, c12, c0, wha8_sb, wfh8_sb)
            emit_b_item()
            emit_b_item()
            if c0 == 0:
                xt_q[(11, 0)] = load_xt_a(11, 0)
        # level 11
        h11, c11 = alloc_hc(hcA, 256)
        hs8, chh8 = make_hs8_chh8(h12, 512)
        csum_cell(pAd, 256, xt_q[(11, 0)], hs8, chh8, c12, 0,
                  h11, c11, 0, wha8_sb, wfh8_sb)
        while b_state["idx"] <= 10:
            emit_b_item()
        close_pool("pAdeep")
        close_pool("hcA12")

        # xg batch for shallow cols (phase A)
        pBd = open_pool("pBdeep", bufs=2)
        pBatchA = open_pool("pBatchA", bufs=2)
        xgshA = pBatchA.tile([128, 16, SH_COLS], H, tag="xgsh", bufs=1)
        for j in range(16):
            ps = pp2.tile([128, SH_COLS], F32, tag="ps_F", name="psb")
            if 8 <= j < 12:  # u gate needs the 1.5-term compensation
                mm_xa15(ps[:], xTsh8, WXA_BLK[j])
            else:  # i, o, f tolerate naive fp8 (sim: relA ~1e-2)
                mm_xa8(ps[:], xTsh8[:, :, 0, :], WXA_BLK[j])
            nc.scalar.activation(out=xgshA[:, j, :], in_=ps[:],
                                 func=AFT.Identity, bias=ba_sb[:, j:j + 1],
                                 scale=IWS)

        # ---- B-deep emitters (interleaved with A-shallow below) ----
        bd = {"h8": b_state["h8"], "c": b_state["c"], "xt": None}

        def bdeep_load_xt(l, c0, W):
            # all triggers ride the sync queue: a trigger stalled on a
            # buffer-free would block its queue, and gpsimd carries the
            # latency-critical h8 dup copies for the level chain
            xt0 = pBd.tile([128, KCX // 2, W], H, tag="xtB", bufs=3,
                           name="xt0")
            xt1 = pBd.tile([128, KCX // 2, W], H, tag="xtB", bufs=3,
                           name="xt1")
            nc.sync.dma_start(
                out=xt0[:], in_=xp_view(XO_B[(l, c0, 0)], KCX // 2 * W,
                                        KCX // 2))
            nc.sync.dma_start(
                out=xt1[:], in_=xp_view(XO_B[(l, c0, 1)], KCX // 2 * W,
                                        KCX // 2))
            segs = []
            for s in range(W // 256):
                t8 = pBd.tile([128, KCX, 256], F8, tag="xt8B", bufs=5,
                              name="xt8")
                nc.sync.dma_start(
                    out=t8[:],
                    in_=xp8_view(XO_A[(l, c0 + s * 256)], KCX * 256))
                segs.append(t8)
            return (xt0, xt1, segs)

        def bdeep_mc(l, c0, mc, W, hb, h8b, cb, h8_pv, c_pv):
            last = l == 13
            xt0, xt1, segs = bd["xt"]
            # i, f, o gates: naive fp8; u: fp16 (x64 weights). All psums are
            # 64x; the recurrence GEMM accumulates into the x-projection psum
            # (parent h stored child-duplicated) so each gate activation
            # reads ONE finished psum with bias + 1/64 scale in one shot.
            g3 = pBd.tile([128, 3, W], H, tag="g3D", bufs=2, name="g3")
            for gn, j in ((0, mc), (1, 8 + mc), (2, 4 + mc)):  # i, f, o
                psx = pp4.tile([128, W], F32, tag="ps_g", name="psx")
                mm_bd8(psx[:], segs, j, c0, h8_pv)
                nc.scalar.activation(out=g3[:, gn, :], in_=psx[:],
                                     func=AFT.Sigmoid,
                                     bias=bb_sb[:, j:j + 1], scale=IWS)
            ju = 12 + mc  # u via fp16 x-projection
            psx = pp4.tile([128, W], F32, tag="ps_g", name="psx")
            mm_bd16(psx[:], xt0, xt1, ju, c0, W, h8_pv)
            gu = pBd.tile([128, W], H, tag="gD_u", bufs=2, name="gu")
            nc.scalar.activation(out=gu[:], in_=psx[:], func=AFT.Tanh,
                                 bias=bb_sb[:, ju:ju + 1], scale=IWS)
            if last:
                cn = pBd.tile([128, W], F32, tag="cnD", bufs=1, name="cn")
                c_dst = cn[:]
            else:
                c_dst = cb[:, mc, c0:c0 + W]
            p0 = c0 // 2
            pc = c_pv[:, mc, p0:p0 + W // 2]
            t1 = pBd.tile([128, W], F32, tag="t1D", bufs=1, name="t1")
            nc.vector.tensor_mul(
                t1[:].rearrange("p (t two) -> p t two", two=2),
                g3[:, 1, :].rearrange("p (t two) -> p t two", two=2),
                _bcast2(pc))
            nc.vector.tensor_mul(c_dst, g3[:, 0, :], gu[:])
            nc.vector.tensor_add(c_dst, c_dst, t1[:])
            th = pBd.tile([128, W], H, tag="thD", bufs=2, name="th")
            nc.scalar.activation(out=th[:], in_=c_dst, func=AFT.Tanh)
            if last:
                hm = pBd.tile([128, W], H, tag="hD", bufs=2, name="hm")
                nc.vector.tensor_mul(hm[:], g3[:, 2, :], th[:])
                chain_update(pBd, hm[:], 2 ** (l - 3), mc=mc)
            else:
                nc.vector.tensor_mul(hb[:, mc, c0:c0 + W], g3[:, 2, :], th[:])
                # parent h stored child-duplicated (stride-0 input view) so
                # the next level's recurrence streams straight into its psum
                nc.vector.tensor_copy(
                    out=h8b[:, mc, 2 * c0:2 * c0 + 2 * W].rearrange(
                        "p (t two) -> p t two", two=2),
                    in_=_bcast2(hb[:, mc, c0:c0 + W]))

        # ---- interleave: A-shallow levels zipped with B-deep l11/l12 ----
        pAs = open_pool("pAsh", bufs=2)
        hA, cA = h11, c11
        h11b, c11b = alloc_hc(hcB, 256)
        h12b, c12b = alloc_hc(hcB, 512)
        h11b8 = hcB.tile([128, KCH, 512], F8, tag="h8b11d", bufs=1,
                         name="h11b8")
        h12b8 = hcB.tile([128, KCH, 1024], F8, tag="h8b12d", bufs=1,
                         name="h12b8")

        xt13 = {}
        sched = [("A", 10), ("B11", 0), ("A", 9), ("B11", 1), ("A", 8),
                 ("B11", 2), ("A", 7), ("B11", 3), ("A", 6), ("B12", 0),
                 ("A", 5), ("B12", 1), ("A", 4), ("B12", 2), ("A", 3),
                 ("B12", 3)]
        for kind, v in sched:
            if kind == "A":
                hA, cA = phaseA_level(pAs, hcA, 2 ** (v - 3), L_OFF[v],
                                      xgshA, hA, cA, wha_sb, wfh_sb)
            elif kind == "B11":
                if v == 0:
                    bd["xt"] = bdeep_load_xt(11, 0, 256)
                bdeep_mc(11, 0, v, 256, h11b, h11b8, c11b, bd["h8"],
                         bd["c"])
                if v == 3:
                    chain_update(pBd, h11b[:, :, 0:256], 256)
            else:
                if v == 0:
                    bd["xt"] = bdeep_load_xt(12, 0, 512)
                if v == 1:
                    xt13[0] = bdeep_load_xt(13, 0, 512)
                bdeep_mc(12, 0, v, 512, h12b, h12b8, c12b, h11b8, c11b)
                if v == 3:
                    chain_update(pBd, h12b[:, :, 0:512], 512)

        # stage this core's level-3 (h, c) for the host-side tree top
        nc.vector.tensor_copy(out=stage[:, 0:4], in_=hA[:, :, 0])
        nc.vector.tensor_copy(out=stage[:, 4:8], in_=cA[:, :, 0])

        close_pool("pAsh")
        close_pool("pBatchA")

        # ---- B-deep level 13 (storeless) ----
        for c0 in (0, 512):
            bd["xt"] = xt13[c0]
            for mc in range(4):
                if c0 == 0 and mc == 2:
                    xt13[512] = bdeep_load_xt(13, 512, 512)
                bdeep_mc(13, c0, mc, 512, None, None, None, h12b8, c12b)
        close_pool("pBdeep")
        close_pool("hcA")
        close_pool("pBsh")
        close_pool("hcB")

        # final output: [h3 | c3 | cmax] transposed on the PE so the DRAM
        # write is 12 contiguous 512B descriptors.
        nc.vector.tensor_copy(out=stage[:, 8:12], in_=cmax[:])
        psT = pp4.tile([128, 128], F32, tag="ps_g", name="psT")
        nc.tensor.matmul(psT[0:12, :], stage[:], ident_sb[:],
                         is_transpose=True)
        stageT = pT.tile([128, 128], F32, tag="stageT", bufs=1)
        nc.scalar.activation(out=stageT[0:12, :], in_=psT[0:12, :],
                             func=AFT.Identity)
        nc.sync.dma_start(
            out=out.ap()[0, :].rearrange("(q n) -> q n", q=12),
            in_=stageT[0:12, :])

        close_pool("pTop")
        close_pool("pp2")
        close_pool("pp4")
        close_pool("persist")

    nc.compile()
    return nc


def _host_inputs(inputs, ifoux_w, ifoux_b, ious_w, ious_b, fh_w, fh_b,
                 iofux_w, iofux_b, iofuh_w, iofuh_b):
    """Build the 8 per-core input maps (host-side sharding / layout only)."""
    f32 = np.float32
    inputs = np.asarray(inputs, f32)
    m = MEM

    import ml_dtypes
    E4 = ml_dtypes.float8_e4m3
    f16 = np.float16

    def pk(a):
        """[K, M] -> partition-major [128, (K/128)*M] (fat-DMA layout)."""
        k, mm_ = a.shape
        return np.ascontiguousarray(
            a.reshape(k // 128, 128, mm_).transpose(1, 0, 2).reshape(128, -1))

    def hl8(w):
        wt = np.ascontiguousarray(np.asarray(w, f32).T) * WS
        hi = wt.astype(E4)
        lo = (wt - hi.astype(f32)).astype(E4)
        return hi, lo

    wxa8h, wxa8l_ = hl8(ifoux_w)
    wxa8i = pk(wxa8h[:, 0:m])
    wxa8f = pk(wxa8h[:, m:2 * m])
    wxa8ou = pk(wxa8h[:, 2 * m:4 * m])
    wxa8l = pk(wxa8l_)
    wha = pk(np.asarray(ious_w, f32).T.astype(f16))
    wfhT = pk(np.asarray(fh_w, f32).T.astype(f16))
    wha8 = pk((np.asarray(ious_w, f32).T * WS).astype(E4))
    wfh8 = pk((np.asarray(fh_w, f32).T * WS).astype(E4))
    wxbT = np.asarray(iofux_w, f32).T  # [IN, 4m], blocks i,o,f,u
    wxb8 = pk((np.concatenate([wxbT[:, 0:m], wxbT[:, 2 * m:3 * m],
                               wxbT[:, m:2 * m]], axis=1) * WS).astype(E4))
    wxbou = pk((wxbT[:, 3 * m:4 * m] * WS).astype(f16))
    whb8 = pk((np.asarray(iofuh_w, f32).T * WS).astype(E4))

    ifoux_b = np.asarray(ifoux_b, f32)
    ious_b = np.asarray(ious_b, f32)
    fh_b = np.asarray(fh_b, f32)
    # phase-A folded biases in j-order i,o,u,f
    ba = np.concatenate([
        ifoux_b[0:m] + ious_b[0:m],                  # i
        ifoux_b[2 * m:3 * m] + ious_b[m:2 * m],      # o
        ifoux_b[3 * m:4 * m] + ious_b[2 * m:3 * m],  # u
        ifoux_b[m:2 * m] + fh_b,                     # f (+ fh bias)
    ])
    ba = np.ascontiguousarray(ba.reshape(16, 128).T)
    bb = np.asarray(iofux_b, f32) + np.asarray(iofuh_b, f32)
    bb = np.ascontiguousarray(bb.reshape(16, 128).T)
    bb64 = np.ascontiguousarray(bb * np.float32(WS))
    identity = np.eye(128, dtype=f32)

    in_maps = []
    for k in range(NCORES):
        idx = [np.arange(7)]
        for l in range(3, DEPTH):
            w = 2 ** (l - 3)
            idx.append((2 ** l - 1) + k * w + np.arange(w))
        idx = np.concatenate(idx)
        xf = inputs[idx].T                             # [IN, NCOLS] f32
        xk = xf.astype(np.float16)
        xr = xk.reshape(KCX, 128, NCOLS)               # [kc, p, n]
        xr8 = xf.astype(E4).reshape(KCX, 128, NCOLS)
        x8lo = (xf - xf.astype(E4).astype(f32)).astype(E4)
        xr8l = x8lo.reshape(KCX, 128, NCOLS)
        xp = np.empty((128, XPACK_LEN), np.float16)
        xp8 = np.empty((128, XPACK8_LEN), E4)

        def seg2p(seg):  # [kc', p, w] -> [p, kc'*w]
            kcn, _, w = seg.shape
            return seg.transpose(1, 0, 2).reshape(128, kcn * w)

        xp[:, XO_SH:XO_SH + KCX * SH_COLS] = seg2p(xr[:, :, 0:SH_COLS])
        for (lv, c0, h), off in XO_B.items():
            base = L_OFF[lv] + c0
            w = 256 if lv == 11 else 512
            xp[:, off:off + 4 * w] = seg2p(xr[4 * h:4 * h + 4, :,
                                              base:base + w])
        for (lv, c0), off in XO_A.items():
            base = L_OFF[lv] + c0
            xp8[:, off:off + KCX * 256] = seg2p(xr8[:, :, base:base + 256])
        xsh8h = np.ascontiguousarray(seg2p(xr8[:, :, 0:SH_COLS]))
        xsh8l = np.ascontiguousarray(seg2p(xr8l[:, :, 0:SH_COLS]))
        xp = np.ascontiguousarray(xp)
        xp8 = np.ascontiguousarray(xp8)
        sel = np.zeros((128, 4), f32)
        sel[:, k // 2] = 1.0
        in_maps.append({
            "xpack": xp, "xpack8": xp8, "xsh8h": xsh8h, "xsh8l": xsh8l,
            "wxa8i": wxa8i, "wxa8ou": wxa8ou, "wxa8f": wxa8f,
            "wxa8l": wxa8l, "wha": wha, "wfh": wfhT,
            "wha8": wha8, "wfh8": wfh8, "wxb8": wxb8, "wxbou": wxbou,
            "whb8": whb8, "ba": ba, "bb": bb, "bb64": bb64, "sel3": sel,
            "ident": identity,
        })
    return in_maps


def _host_top(h3, c3, inputs, ifoux_w, ifoux_b, ious_w, ious_b, fh_w, fh_b):
    """fp32 ChildSum over the top 3 levels (nodes 0..6) from the gathered
    level-3 children (nodes 7..14)."""
    f32 = np.float32
    m = MEM

    def sig(v):
        return 1.0 / (1.0 + np.exp(-v))

    xg = (np.asarray(inputs[0:7], f32) @ np.asarray(ifoux_w, f32).T
          + np.asarray(ifoux_b, f32))
    ix, fx = xg[:, :m], xg[:, m:2 * m]
    ox, ux = xg[:, 2 * m:3 * m], xg[:, 3 * m:]
    wi = np.asarray(ious_w, f32)
    bi = np.asarray(ious_b, f32)
    wf = np.asarray(fh_w, f32)
    bf = np.asarray(fh_b, f32)
    h = np.zeros((7, m), f32)
    c = np.zeros((7, m), f32)
    ch_h = np.asarray(h3, f32).reshape(4, 2, m)
    ch_c = np.asarray(c3, f32).reshape(4, 2, m)
    for lvl in (2, 1, 0):
        idx = np.arange(2 ** lvl - 1, 2 ** (lvl + 1) - 1)
        if lvl < 2:
            ch = np.stack([2 * idx + 1, 2 * idx + 2], axis=1)
            ch_h = h[ch]
            ch_c = c[ch]
        hsum = ch_h.sum(axis=1)
        iou = hsum @ wi.T + bi
        i = sig(ix[idx] + iou[:, :m])
        o = sig(ox[idx] + iou[:, m:2 * m])
        u = np.tanh(ux[idx] + iou[:, 2 * m:])
        f = sig(ch_h @ wf.T + bf + fx[idx][:, None, :])
        c[idx] = i * u + (f * ch_c).sum(axis=1)
        h[idx] = o * np.tanh(c[idx])
    return h[0]


def _get_prog():
    global _PROG
    if _PROG is None:
        _PROG = build()
    return _PROG


def kernel(inputs, ifoux_w, ifoux_b, ious_w, ious_b, fh_w, fh_b,
           iofux_w, iofux_b, iofuh_w, iofuh_b, depth=DEPTH, **_unused):
    assert int(depth) == DEPTH, f"kernel hardcodes depth={DEPTH}"
    nc = _get_prog()
    in_maps = _host_inputs(inputs, ifoux_w, ifoux_b, ious_w, ious_b,
                           fh_w, fh_b, iofux_w, iofux_b, iofuh_w, iofuh_b)
    res = run_bass_kernel_spmd(nc, in_maps, list(range(NCORES)))
    outs = [res.results[k]["out"][0] for k in range(NCORES)]
    h3 = np.stack([o[0:MEM] for o in outs])          # nodes 7..14
    c3 = np.stack([o[MEM:2 * MEM] for o in outs])
    cmax = np.max(np.stack([o[2 * MEM:] for o in outs]), axis=0)
    frep = _host_top(h3, c3, inputs, ifoux_w, ifoux_b, ious_w, ious_b,
                     fh_w, fh_b)
    return np.concatenate([frep, cmax])[None, :].astype(np.float32)


if __name__ == "__main__":
    import sys
    if len(sys.argv) > 1 and sys.argv[1] == "emit":
        real_compile = bacc.Bacc.compile
        bacc.Bacc.compile = lambda self: None
        try:
            build()
            print("emit OK")
        finally:
            bacc.Bacc.compile = real_compile
